# revision 29
# baseline (speedup 1.0000x reference)
"""EGNN (gnn_message_passing) Trainium2 Bass kernel, v3.

v2 -> v3 changes, driven by the TimelineSim cost model (matmul cost =
OUT free size x cycles/row, independent of K; ACT/DVE cost = max free
size; DVE 2x perf mode needs all-2-byte packed operands):
- b2 enters stage2 via an ones-row in h (ph row 50/114 is forced to 16
  through the b1pad path; silu(16) rounds to exactly 16 in bf16; W2 row
  50/114 = b2/16).  Kills the 8 K=1 b2 matmuls per chunk-pair (-32K PE
  rows per mol-layer).
- stage1 is 2 matmuls per slot instead of 3: the per-j term (W1fj.f_j)
  and the dist term share one matmul against a packed [16,128,N] rhs
  tile RD whose rows 0:12 are feats replicated over the 128 slots
  (layer 0 comes replicated from DRAM; layer 1 is an SBUF->SBUF
  broadcast DMA in 8 pieces) and rows 12:16 are the hi/lo dist rows.
- ACT ops are [128,1024] spanning 2 PSUM banks: one shared 3-buf PSUM
  pool (6 banks) alternates stage1 ph / stage2 pm tiles, so both silus
  run at 1024 free (halves the per-op ACT overhead).
- gate z uses a fp16 fold tree (mult, 2 folds, reduce-16) instead of
  mult + full reduce: ~340ns less DVE per pair.
Layout is otherwise v2's: slots are i-pairs (2 dest nodes x 256 j) with
the two parities in partition quadrants 0:50 / 64:114; stage2 is
transposed into edge-major [128 j, 64 f] blocks; gating and
j-aggregation ride the PE as K=128, out-free-1 matmuls.
"""

import numpy as np
import ml_dtypes

import concourse.bass as bass
import concourse.bacc as bacc
import concourse.mybir as mybir
from concourse.tile import TileContext
from concourse.bass_utils import run_bass_kernel_spmd

F32 = mybir.dt.float32
F16 = mybir.dt.float16
BF16 = mybir.dt.bfloat16
AF = mybir.ActivationFunctionType
ALU = mybir.AluOpType
X = mybir.AxisListType.X

LIP = 0.909
NCORES = 8
BM = 2            # molecules per core
N = 256           # nodes per molecule
L = 2             # layers
D = 12            # feature dim
M = 64            # message dim
EH = 50           # edge hidden
Q = 64            # partition quadrant stride for the j-odd half
NSLOT = 128       # i-pair slots per molecule-layer
NPAIR = 32        # slot quads (4 slots / [128,1024] tile)
NMEB = 6          # me sbuf ring depth (pairs)
CONE = 16.0       # ones-row magnitude: silu(16) == 16 exactly in bf16

WBF_SPEC = [
    ("s1w", 16, L * 128), ("w1fjE", D, L * 128), ("w1fjO", D, L * 128),
    ("b1pad", 1, L * 128), ("w2q", 128, L * 128), ("wgrep", 128, L * 1024),
    ("i128b", 128, 128), ("c12", D, 1),
]
WF32_SPEC = [
    ("gbh", 128, L), ("lng", D, L), ("lnb", D, L), ("nw1a", D, L * 24),
    ("nw1b", M, L * 24), ("nb1", 24, L), ("nw2", 24, L * D),
    ("nb2", D, L), ("mw1", D, M), ("mb1", M, 1), ("mw2", M, 2),
    ("mb2", 2, 1), ("i128f", 128, 128),
]


def _offsets(spec):
    out, off = {}, 0
    for nm, p, w in spec:
        out[nm] = (p, w, off)
        off += w
    return out, off


WBF_OFF, WBF_X = _offsets(WBF_SPEC)
WF32_OFF, WF32_X = _offsets(WF32_SPEC)


def build_nc():
    nc = bacc.Bacc("TRN2", target_bir_lowering=False, debug=False)

    feats0 = nc.dram_tensor("feats0", [BM, D, N], F32, kind="ExternalInput")
    rdin = nc.dram_tensor("rdin", [BM, 16, 128, N], BF16,
                          kind="ExternalInput")
    mask12 = nc.dram_tensor("mask12", [BM, D, N], F32, kind="ExternalInput")
    wbf = nc.dram_tensor("wbf", [128, WBF_X], BF16, kind="ExternalInput")
    wf32 = nc.dram_tensor("wf32", [128, WF32_X], F32, kind="ExternalInput")
    out = nc.dram_tensor("out", [BM, N, 2, 6], F32, kind="ExternalOutput")

    with TileContext(nc) as tc:
        with (
            tc.tile_pool(name="singles", bufs=1) as S,
            tc.tile_pool(name="mol", bufs=3) as MP,
            tc.tile_pool(name="rdp", bufs=2) as RDP,
            tc.tile_pool(name="lay", bufs=3) as LP,
            tc.tile_pool(name="pbig", bufs=3, space="PSUM") as PB,
            tc.tile_pool(name="psml", bufs=1, space="PSUM") as PS,
            tc.tile_pool(name="pg", bufs=1, space="PSUM") as PG,
        ):
            wbf_s = S.tile([128, WBF_X], BF16, tag="wbf", name="wbf")
            nc.sync.dma_start(out=wbf_s, in_=wbf[:, :])
            wf32_s = S.tile([128, WF32_X], F32, tag="wf32", name="wf32")
            nc.sync.dma_start(out=wf32_s, in_=wf32[:, :])

            def bsl(nm):
                p, w, off = WBF_OFF[nm]
                return wbf_s[0:p, off:off + w]

            def fsl(nm):
                p, w, off = WF32_OFF[nm]
                return wf32_s[0:p, off:off + w]

            s1w_s = bsl("s1w")
            w1fjE_s = bsl("w1fjE")
            w1fjO_s = bsl("w1fjO")
            b1pad_s = bsl("b1pad")
            w2q_s = bsl("w2q")
            wgrep_s = bsl("wgrep")
            i128b_s = bsl("i128b")
            c12_s = bsl("c12")
            gbh_s = fsl("gbh")
            lng_s = fsl("lng")
            lnb_s = fsl("lnb")
            nw1a_s = fsl("nw1a")
            nw1b_s = fsl("nw1b")
            nb1_s = fsl("nb1")
            nw2_s = fsl("nw2")
            nb2_s = fsl("nb2")
            mw1_s = fsl("mw1")
            mb1_s = fsl("mb1")
            mw2_s = fsl("mw2")
            mb2_s = fsl("mb2")
            i128f_s = fsl("i128f")

            onesr = S.tile([1, 128], BF16, tag="onesr")
            nc.vector.memset(onesr, 1.0)
            eps = S.tile([1, 1], F32, tag="eps")
            nc.vector.memset(eps, 1e-5)

            h_bufs = [S.tile([128, 1024], BF16, tag=f"h{k}", name=f"h{k}")
                      for k in range(3)]
            me_bufs = [S.tile([128, 1024], BF16, tag=f"me{k}", name=f"me{k}")
                       for k in range(NMEB)]
            tmp_bufs = [S.tile([128, 1024], F16, tag=f"tmp{k}",
                               name=f"tmp{k}") for k in range(2)]
            fd1_bufs = [S.tile([128, 512], F16, tag=f"fd1{k}",
                               name=f"fd1{k}") for k in range(2)]
            fd2_bufs = [S.tile([128, 256], F16, tag=f"fd2{k}",
                               name=f"fd2{k}") for k in range(2)]
            zb_bufs = [S.tile([128, 64], F32, tag=f"zb{k}", name=f"zb{k}")
                       for k in range(2)]
            th_bufs = [S.tile([128, 64], BF16, tag=f"th{k}", name=f"th{k}")
                       for k in range(2)]
            tp_bufs = [S.tile([128, 64], BF16, tag=f"tp{k}", name=f"tp{k}")
                       for k in range(2)]
            opad = S.tile([2, N, 6], F32, tag="opad")
            nc.vector.memset(opad, 0.0)

            def rear3(t, b):
                # [128, b*w] tile viewed as [128, b, w]
                return t.rearrange("p (b f) -> p b f", b=b)

            for mol in range(BM):
                feats = MP.tile([D, N], F32, tag="feats")
                nc.sync.dma_start(out=feats, in_=feats0[mol])
                rd = RDP.tile([16, 128, N], BF16, tag="rd")
                nc.sync.dma_start(out=rd, in_=rdin[mol])
                msk = MP.tile([D, N], F32, tag="msk")
                nc.sync.dma_start(out=msk, in_=mask12[mol])

                for lay in range(L):
                    fb = LP.tile([D, N], BF16, tag="fb")
                    nc.vector.tensor_copy(out=fb, in_=feats)
                    if lay > 0:
                        # refresh the replicated-feats rows of RD
                        for k in range(8):
                            nc.sync.dma_start(
                                out=rd[0:D, 16 * k:16 * (k + 1), :],
                                in_=bass.AP(
                                    tensor=fb.tensor, offset=fb.offset,
                                    ap=[list(fb.ap[0]), [0, 16], [1, N]]))

                    # ---- per-i constant matrix fjwb / fjT ----
                    ps_fj = PS.tile([128, 128], F32, tag="pa")
                    fe = fb.rearrange("p (c two) -> p two c", two=2)
                    nc.tensor.matmul(
                        ps_fj, lhsT=w1fjE_s[:, lay * 128:(lay + 1) * 128],
                        rhs=fe[:, 0, :], start=True, stop=False)
                    nc.tensor.matmul(
                        ps_fj, lhsT=w1fjO_s[:, lay * 128:(lay + 1) * 128],
                        rhs=fe[:, 1, :], start=False, stop=False)
                    nc.tensor.matmul(
                        ps_fj, lhsT=b1pad_s[:, lay * 128:(lay + 1) * 128],
                        rhs=onesr, start=False, stop=True)
                    fjwb = LP.tile([128, 128], F32, tag="fjwb")
                    nc.vector.tensor_copy(out=fjwb, in_=ps_fj)
                    ps_ft = PS.tile([128, 128], F32, tag="pa")
                    nc.tensor.transpose(ps_ft, fjwb, i128f_s)
                    fjT = LP.tile([128, 128], BF16, tag="fjT")
                    nc.vector.tensor_copy(out=fjT, in_=ps_ft)

                    # ---- LayerNorm of feats (feeds node MLP later) ----
                    ps_mu = PS.tile([1, N], F32, tag="pa")
                    nc.tensor.matmul(ps_mu, lhsT=c12_s, rhs=fb,
                                     start=True, stop=True)
                    stat = LP.tile([1, 2 * N], BF16, tag="stat")
                    nc.vector.tensor_copy(out=stat[:, 0:N], in_=ps_mu)
                    ps_bm = PS.tile([D, N], F32, tag="pa")
                    nc.tensor.matmul(ps_bm, lhsT=onesr[:, 0:D],
                                     rhs=stat[:, 0:N], start=True, stop=True)
                    ctr = LP.tile([D, N], BF16, tag="ctr")
                    nc.vector.tensor_sub(ctr, fb, ps_bm)
                    sqc = LP.tile([D, N], BF16, tag="sqc")
                    nc.vector.tensor_mul(sqc, ctr, ctr)
                    ps_v = PS.tile([1, N], F32, tag="pa")
                    nc.tensor.matmul(ps_v, lhsT=c12_s, rhs=sqc,
                                     start=True, stop=True)
                    sd = LP.tile([1, N], F32, tag="sd")
                    nc.scalar.activation(sd, ps_v, AF.Sqrt, bias=eps[:, 0:1])
                    with nc.allow_low_precision(reason="bf16 rstd"):
                        nc.vector.reciprocal(stat[:, N:2 * N], sd)
                    ps_br = PS.tile([D, N], F32, tag="pa")
                    nc.tensor.matmul(ps_br, lhsT=onesr[:, 0:D],
                                     rhs=stat[:, N:2 * N], start=True,
                                     stop=True)
                    nrm = LP.tile([D, N], BF16, tag="nrm")
                    nc.vector.tensor_mul(nrm, ctr, ps_br)
                    normed = LP.tile([D, N], F32, tag="normed")
                    nc.vector.tensor_scalar(
                        out=normed, in0=nrm,
                        scalar1=lng_s[:, lay:lay + 1],
                        scalar2=lnb_s[:, lay:lay + 1],
                        op0=ALU.mult, op1=ALU.add)

                    s1w_l = s1w_s[:, lay * 128:(lay + 1) * 128]
                    w2q_l = w2q_s[:, lay * 128:(lay + 1) * 128]
                    wg_l = wgrep_s[:, lay * 1024:(lay + 1) * 1024]
                    ps_mg = PG.tile([M, N], F32, tag="magg")

                    # software-pipelined pair loop: stage2+tail of pair k
                    # is emitted after stage1+h-act of pair k+1 so the ACT
                    # queue alternates h-act(k+1) / me-act(k) without
                    # waiting on the intervening stage2 matmuls.
                    def emit_stage1(pp):
                        ph = PB.tile([128, 1024], F32, tag="big")
                        for hl in range(4):
                            s = 4 * pp + hl
                            sl = slice(hl * N, (hl + 1) * N)
                            nc.tensor.matmul(ph[:, sl], lhsT=s1w_l,
                                             rhs=rd[:, s, :],
                                             start=True, stop=False)
                            icol = i128b_s[:, s:s + 1]
                            ind_ap = bass.AP(
                                tensor=icol.tensor, offset=icol.offset,
                                ap=[list(icol.ap[0]), [0, N]])
                            nc.tensor.matmul(ph[:, sl], lhsT=fjT,
                                             rhs=ind_ap,
                                             start=False, stop=True)
                        h = h_bufs[pp % 3]
                        nc.scalar.activation(h, ph, AF.Silu)
                        return h

                    def emit_stage2(pp, h):
                        # 8 double-block matmuls: full-128 lhsT against the
                        # block-diagonal w2 pair -> even-i block in cols
                        # 0:64, odd-i in 64:128 (lhsT base stays 0:
                        # unchained matmuls with alternating lhsT partition
                        # bases hang the device).
                        pm = PB.tile([128, 1024], F32, tag="big")
                        for hl in range(4):
                            for ib in range(2):   # j half-block
                                db = hl * 2 + ib
                                po = pm[:, db * 128:(db + 1) * 128]
                                hs = h[:, hl * N + ib * 128:
                                       hl * N + (ib + 1) * 128]
                                nc.tensor.matmul(po, lhsT=hs, rhs=w2q_l,
                                                 start=True, stop=True)
                        return pm

                    def emit_tail(pp, pm):
                        me = me_bufs[pp % NMEB]
                        nc.scalar.activation(me, pm, AF.Silu)
                        # gate z: mult + fold tree (fp16)
                        tmp = tmp_bufs[pp % 2]
                        nc.vector.tensor_mul(tmp, me, wg_l)
                        t3 = rear3(tmp, 16)
                        fd1 = fd1_bufs[pp % 2]
                        nc.vector.tensor_tensor(
                            out=rear3(fd1, 16), in0=t3[:, :, 0:32],
                            in1=t3[:, :, 32:64], op=ALU.add)
                        f13 = rear3(fd1, 16)
                        fd2 = fd2_bufs[pp % 2]
                        nc.vector.tensor_tensor(
                            out=rear3(fd2, 16), in0=f13[:, :, 0:16],
                            in1=f13[:, :, 16:32], op=ALU.add)
                        zb = zb_bufs[(pp // 4) % 2]
                        nc.vector.tensor_reduce(
                            out=zb[:, (pp % 4) * 16:(pp % 4) * 16 + 16],
                            in_=rear3(fd2, 16), op=ALU.add, axis=X)
                        if pp % 4 == 3:
                            # gate + j-aggregation for this sg
                            sgi = pp // 4
                            zbs = zb_bufs[sgi % 2]
                            th = th_bufs[sgi % 2]
                            nc.scalar.activation(
                                th, zbs, AF.Tanh,
                                bias=gbh_s[:, lay:lay + 1], scale=0.5)
                            tp = tp_bufs[sgi % 2]
                            nc.vector.tensor_scalar_add(tp, th, 1.0)
                            pp0 = pp - 3
                            for q4 in range(4):
                                meq = me_bufs[(pp0 + q4) % NMEB]
                                for hl in range(4):
                                    for jj in range(2):
                                        i = 8 * (pp0 + q4) + 2 * hl + jj
                                        for ib in range(2):
                                            b = hl * 4 + ib * 2 + jj
                                            col = q4 * 16 + b
                                            nc.tensor.matmul(
                                                ps_mg[:, i:i + 1],
                                                lhsT=meq[:, b * M:
                                                         (b + 1) * M],
                                                rhs=tp[:, col:col + 1],
                                                start=(ib == 0),
                                                stop=(ib == 1))

                    pend = None
                    for pp in range(NPAIR):
                        h = emit_stage1(pp)
                        if pend is not None:
                            emit_tail(pend[0], pend[1])
                        pm = emit_stage2(pp, h)
                        pend = (pp, pm)
                    emit_tail(pend[0], pend[1])

                    magg = LP.tile([M, N], F32, tag="magg")
                    nc.vector.tensor_copy(out=magg, in_=ps_mg)

                    # ---- node MLP + residual ----
                    ps_z1 = PS.tile([24, N], F32, tag="pa")
                    nc.tensor.matmul(ps_z1,
                                     lhsT=nw1a_s[:, lay * 24:(lay + 1) * 24],
                                     rhs=normed, start=True, stop=False)
                    nc.tensor.matmul(ps_z1,
                                     lhsT=nw1b_s[:, lay * 24:(lay + 1) * 24],
                                     rhs=magg, start=False, stop=True)
                    s1 = LP.tile([24, N], F32, tag="s1")
                    nc.scalar.activation(s1, ps_z1, AF.Silu,
                                         bias=nb1_s[:, lay:lay + 1])
                    ps_z2 = PS.tile([D, N], F32, tag="pa")
                    nc.tensor.matmul(ps_z2,
                                     lhsT=nw2_s[:, lay * D:(lay + 1) * D],
                                     rhs=s1, start=True, stop=True)
                    feats_new = MP.tile([D, N], F32, tag="feats")
                    nc.vector.scalar_tensor_tensor(
                        out=feats_new, in0=ps_z2,
                        scalar=nb2_s[:, lay:lay + 1], in1=feats,
                        op0=ALU.add, op1=ALU.add)
                    feats = feats_new

                # ---- final head ----
                fmask = MP.tile([D, N], F32, tag="fmask")
                nc.vector.tensor_mul(fmask, feats, msk)
                ps_r = PS.tile([M, N], F32, tag="pa")
                nc.tensor.matmul(ps_r, lhsT=mw1_s, rhs=fmask,
                                 start=True, stop=True)
                r1 = MP.tile([M, N], F32, tag="r1")
                nc.scalar.activation(r1, ps_r, AF.Relu, bias=mb1_s[:, 0:1])
                ps_o = PS.tile([2, N], F32, tag="pa")
                nc.tensor.matmul(ps_o, lhsT=mw2_s, rhs=r1,
                                 start=True, stop=True)
                nc.vector.tensor_scalar_add(opad[:, :, 0:1], ps_o,
                                            mb2_s[:, 0:1])
                nc.sync.dma_start(
                    out=out[mol].rearrange("n c k -> c n k"), in_=opad)

    nc.finalize()
    return nc


_NC = None


def _get_nc():
    global _NC
    if _NC is None:
        _NC = build_nc()
    return _NC


def _bf(a):
    return np.ascontiguousarray(np.asarray(a, np.float32).astype(
        ml_dtypes.bfloat16))


def _prep_maps(x, mask, edge_w1, edge_b1, edge_w2, edge_b2, gate_w, gate_b,
               ln_g, ln_b, node_w1, node_b1, node_w2, node_b2,
               mlp_w1, mlp_b1, mlp_w2, mlp_b2):
    f = np.float32
    x = np.asarray(x, f)
    maskf = np.asarray(mask, f)
    ew1 = np.asarray(edge_w1, f)          # [L, 25, 50]
    eb1 = np.asarray(edge_b1, f)          # [L, 50]
    ew2 = np.asarray(edge_w2, f) * LIP    # [L, 50, 64]
    eb2 = np.asarray(edge_b2, f)          # [L, 64]
    gw = np.asarray(gate_w, f) * LIP      # [L, 64, 1]
    gb = np.asarray(gate_b, f)            # [L, 1]

    # slots are i-pairs: the re-added per-i term uses ew1 rows 0:D, the
    # per-j matmul term uses ew1 rows D:2D (feats_j against W1's fj block)
    w1fj_h = ew1[:, D:2 * D, :]           # [L, 12, 50] per-j weights
    w1fi_h = ew1[:, 0:D, :]               # [L, 12, 50] per-i weights
    w1d = ew1[:, 2 * D, :]                # [L, 50]

    s1w_h = np.zeros((16, L, 128), f)
    w1fjE_h = np.zeros((D, L, 128), f)
    w1fjO_h = np.zeros((D, L, 128), f)
    b1pad_h = np.zeros((1, L, 128), f)
    w2q_h = np.zeros((128, L, 128), f)
    wgrep_h = np.zeros((128, L, 1024), f)
    gbh_h = np.zeros((128, L), f)
    for l in range(L):
        # stage1 combined lhsT: rows 0:12 feats_j weights (both quadrants),
        # rows 12:16 dist hi/lo x quadrant
        s1w_h[0:D, l, 0:EH] = w1fj_h[l]
        s1w_h[0:D, l, Q:Q + EH] = w1fj_h[l]
        s1w_h[12, l, 0:EH] = w1d[l]
        s1w_h[13, l, 0:EH] = w1d[l]
        s1w_h[14, l, Q:Q + EH] = w1d[l]
        s1w_h[15, l, Q:Q + EH] = w1d[l]
        # per-i matrix path (fjwb): i-even / i-odd quadrants + bias; col
        # EH/Q+EH carries the ones-row magnitude CONE for the b2 trick
        w1fjE_h[:, l, 0:EH] = w1fi_h[l]
        w1fjO_h[:, l, Q:Q + EH] = w1fi_h[l]
        b1pad_h[0, l, 0:EH] = eb1[l]
        b1pad_h[0, l, EH] = CONE
        b1pad_h[0, l, Q:Q + EH] = eb1[l]
        b1pad_h[0, l, Q + EH] = CONE
        # block-diagonal pair: even-i quadrant rows -> cols 0:64,
        # odd-i quadrant rows -> cols 64:128; row EH/Q+EH carries b2/CONE
        w2q_h[0:EH, l, 0:M] = ew2[l]
        w2q_h[EH, l, 0:M] = eb2[l] / CONE
        w2q_h[Q:Q + EH, l, M:2 * M] = ew2[l]
        w2q_h[Q + EH, l, M:2 * M] = eb2[l] / CONE
        wgrep_h[:, l, :] = np.tile(gw[l, :, 0], (128, 16))
        gbh_h[:, l] = gb[l, 0] * 0.5

    nw1 = np.asarray(node_w1, f)          # [L, 76, 24]
    nw1a_h = np.transpose(nw1[:, 0:D, :], (1, 0, 2))       # [12, L, 24]
    nw1b_h = np.transpose(nw1[:, D:, :] * (LIP * 0.5), (1, 0, 2))
    nw2_h = np.transpose(np.asarray(node_w2, f) * LIP, (1, 0, 2))

    parts = dict(
        s1w=_bf(s1w_h.reshape(16, L * 128)),
        w1fjE=_bf(w1fjE_h.reshape(D, L * 128)),
        w1fjO=_bf(w1fjO_h.reshape(D, L * 128)),
        b1pad=_bf(b1pad_h.reshape(1, L * 128)),
        w2q=_bf(w2q_h.reshape(128, L * 128)),
        wgrep=_bf(wgrep_h.reshape(128, L * 1024)),
        i128b=_bf(np.eye(128, dtype=f)),
        c12=_bf(np.full((D, 1), 1.0 / D, f)),
    )
    partsf = dict(
        gbh=gbh_h,
        lng=np.asarray(ln_g, f).T,
        lnb=np.asarray(ln_b, f).T,
        nw1a=nw1a_h.reshape(D, L * 24),
        nw1b=nw1b_h.reshape(M, L * 24),
        nb1=np.asarray(node_b1, f).T,
        nw2=nw2_h.reshape(24, L * D),
        nb2=np.asarray(node_b2, f).T,
        mw1=np.asarray(mlp_w1, f),
        mb1=np.asarray(mlp_b1, f).reshape(M, 1),
        mw2=np.asarray(mlp_w2, f),
        mb2=np.asarray(mlp_b2, f).reshape(2, 1),
        i128f=np.eye(128, dtype=f),
    )
    wbf_h = np.zeros((128, WBF_X), ml_dtypes.bfloat16)
    for nm, p, w in WBF_SPEC:
        wbf_h[0:p, WBF_OFF[nm][2]:WBF_OFF[nm][2] + w] = parts[nm]
    wf32_h = np.zeros((128, WF32_X), f)
    for nm, p, w in WF32_SPEC:
        wf32_h[0:p, WF32_OFF[nm][2]:WF32_OFF[nm][2] + w] = partsf[nm]
    shared = dict(wbf=wbf_h, wf32=wf32_h)

    in_maps = []
    for core in range(NCORES):
        xs = x[core * BM:(core + 1) * BM]          # [2, 256, 6]
        feats0_h = np.zeros((BM, D, N), f)
        rd_h = np.zeros((BM, 16, 128, N), np.float32)
        m12 = np.zeros((BM, D, N), f)
        for m in range(BM):
            xm = xs[m]                              # [256, 6]
            fcat = np.concatenate([xm, xm], axis=1).T   # [12, 256]
            feats0_h[m] = fcat
            # rows 0:12: layer-0 feats (bf16) replicated over the 128 slots
            fcat_bf = fcat.astype(ml_dtypes.bfloat16).astype(np.float32)
            rd_h[m, 0:D] = np.broadcast_to(fcat_bf[:, None, :],
                                           (D, 128, N))
            nsq = np.sum(xm * xm, axis=1)           # [256]
            dmat = nsq[:, None] + nsq[None, :] - 2.0 * (xm @ xm.T)
            # rows 12:16 (parity, hi/lo): rd[12+2p+q][s, j] = d(2s + p, j)
            dpc = dmat.reshape(128, 2, N).transpose(1, 0, 2)  # [p, s, j]
            dhi = dpc.astype(ml_dtypes.bfloat16).astype(np.float32)
            dlo = dpc - dhi
            rd_h[m, 12] = dhi[0]
            rd_h[m, 13] = dlo[0]
            rd_h[m, 14] = dhi[1]
            rd_h[m, 15] = dlo[1]
            m12[m] = np.broadcast_to(maskf[core * BM + m], (D, N))
        in_maps.append(dict(
            feats0=np.ascontiguousarray(feats0_h), rdin=_bf(rd_h),
            mask12=np.ascontiguousarray(m12),
            **{k: v.copy() for k, v in shared.items()},
        ))
    return in_maps


def kernel(**inputs):
    nc = _get_nc()
    in_maps = _prep_maps(**inputs)
    res = run_bass_kernel_spmd(nc, in_maps, core_ids=list(range(NCORES)))
    out = np.concatenate([r["out"] for r in res.results], axis=0)
    return out.astype(np.float32)


# revision 31
# speedup vs baseline: 1.1488x; 1.1488x over previous
"""EGNN (gnn_message_passing) Trainium2 Bass kernel, v3.

v2 -> v3 changes, driven by the TimelineSim cost model (matmul cost =
OUT free size x cycles/row, independent of K; ACT/DVE cost = max free
size; DVE 2x perf mode needs all-2-byte packed operands):
- b2 enters stage2 via an ones-row in h (ph row 50/114 is forced to 16
  through the b1pad path; silu(16) rounds to exactly 16 in bf16; W2 row
  50/114 = b2/16).  Kills the 8 K=1 b2 matmuls per chunk-pair (-32K PE
  rows per mol-layer).
- stage1 is 2 matmuls per slot instead of 3: the per-j term (W1fj.f_j)
  and the dist term share one matmul against a packed [16,128,N] rhs
  tile RD whose rows 0:12 are feats replicated over the 128 slots
  (layer 0 comes replicated from DRAM; layer 1 is an SBUF->SBUF
  broadcast DMA in 8 pieces) and rows 12:16 are the hi/lo dist rows.
- ACT ops are [128,1024] spanning 2 PSUM banks: one shared 3-buf PSUM
  pool (6 banks) alternates stage1 ph / stage2 pm tiles, so both silus
  run at 1024 free (halves the per-op ACT overhead).
- gate z uses a fp16 fold tree (mult, 2 folds, reduce-16) instead of
  mult + full reduce: ~340ns less DVE per pair.
Layout is otherwise v2's: slots are i-pairs (2 dest nodes x 256 j) with
the two parities in partition quadrants 0:50 / 64:114; stage2 is
transposed into edge-major [128 j, 64 f] blocks; gating and
j-aggregation ride the PE as K=128, out-free-1 matmuls.
"""

import numpy as np
import ml_dtypes

import concourse.bass as bass
import concourse.bacc as bacc
import concourse.mybir as mybir
from concourse.tile import TileContext
from concourse.bass_utils import run_bass_kernel_spmd

F32 = mybir.dt.float32
F16 = mybir.dt.float16
BF16 = mybir.dt.bfloat16
AF = mybir.ActivationFunctionType
ALU = mybir.AluOpType
X = mybir.AxisListType.X

LIP = 0.909
NCORES = 8
BM = 2            # molecules per core
N = 256           # nodes per molecule
L = 2             # layers
D = 12            # feature dim
M = 64            # message dim
EH = 50           # edge hidden
Q = 64            # partition quadrant stride for the j-odd half
NSLOT = 128       # i-pair slots per molecule-layer
NPAIR = 32        # slot quads (4 slots / [128,1024] tile)
NMEB = 6          # me sbuf ring depth (pairs)
CONE = 16.0       # ones-row magnitude: silu(16) == 16 exactly in bf16

WBF_SPEC = [
    ("s1w", 16, L * 128), ("w1fjE", D, L * 128), ("w1fjO", D, L * 128),
    ("b1pad", 1, L * 128), ("w2q", 128, L * 128), ("wgrep", 128, L * 1024),
    ("i128b", 128, 128), ("c12", D, 1),
]
WF32_SPEC = [
    ("gbh", 128, L), ("lng", D, L), ("lnb", D, L), ("nw1a", D, L * 24),
    ("nw1b", M, L * 24), ("nb1", 24, L), ("nw2", 24, L * D),
    ("nb2", D, L), ("mw1", D, M), ("mb1", M, 1), ("mw2", M, 2),
    ("mb2", 2, 1), ("i128f", 128, 128),
]


def _offsets(spec):
    out, off = {}, 0
    for nm, p, w in spec:
        out[nm] = (p, w, off)
        off += w
    return out, off


WBF_OFF, WBF_X = _offsets(WBF_SPEC)
WF32_OFF, WF32_X = _offsets(WF32_SPEC)


def build_nc():
    nc = bacc.Bacc("TRN2", target_bir_lowering=False, debug=False)

    feats0 = nc.dram_tensor("feats0", [BM, D, N], F32, kind="ExternalInput")
    rdin = nc.dram_tensor("rdin", [BM, 16, 128, N], BF16,
                          kind="ExternalInput")
    mask12 = nc.dram_tensor("mask12", [BM, D, N], F32, kind="ExternalInput")
    wbf = nc.dram_tensor("wbf", [128, WBF_X], BF16, kind="ExternalInput")
    wf32 = nc.dram_tensor("wf32", [128, WF32_X], F32, kind="ExternalInput")
    out = nc.dram_tensor("out", [BM, N, 2, 6], F32, kind="ExternalOutput")

    with TileContext(nc) as tc:
        with (
            tc.tile_pool(name="singles", bufs=1) as S,
            tc.tile_pool(name="mol", bufs=3) as MP,
            tc.tile_pool(name="rdp", bufs=2) as RDP,
            tc.tile_pool(name="lay", bufs=3) as LP,
            tc.tile_pool(name="ph", bufs=2, space="PSUM") as PH,
            tc.tile_pool(name="pm", bufs=4, space="PSUM") as PM,
            tc.tile_pool(name="psml", bufs=1, space="PSUM") as PS,
            tc.tile_pool(name="pg", bufs=1, space="PSUM") as PG,
        ):
            wbf_s = S.tile([128, WBF_X], BF16, tag="wbf", name="wbf")
            nc.sync.dma_start(out=wbf_s, in_=wbf[:, :])
            wf32_s = S.tile([128, WF32_X], F32, tag="wf32", name="wf32")
            nc.sync.dma_start(out=wf32_s, in_=wf32[:, :])

            def bsl(nm):
                p, w, off = WBF_OFF[nm]
                return wbf_s[0:p, off:off + w]

            def fsl(nm):
                p, w, off = WF32_OFF[nm]
                return wf32_s[0:p, off:off + w]

            s1w_s = bsl("s1w")
            w1fjE_s = bsl("w1fjE")
            w1fjO_s = bsl("w1fjO")
            b1pad_s = bsl("b1pad")
            w2q_s = bsl("w2q")
            wgrep_s = bsl("wgrep")
            i128b_s = bsl("i128b")
            c12_s = bsl("c12")
            gbh_s = fsl("gbh")
            lng_s = fsl("lng")
            lnb_s = fsl("lnb")
            nw1a_s = fsl("nw1a")
            nw1b_s = fsl("nw1b")
            nb1_s = fsl("nb1")
            nw2_s = fsl("nw2")
            nb2_s = fsl("nb2")
            mw1_s = fsl("mw1")
            mb1_s = fsl("mb1")
            mw2_s = fsl("mw2")
            mb2_s = fsl("mb2")
            i128f_s = fsl("i128f")

            onesr = S.tile([1, 128], BF16, tag="onesr")
            nc.vector.memset(onesr, 1.0)
            eps = S.tile([1, 1], F32, tag="eps")
            nc.vector.memset(eps, 1e-5)

            h_bufs = [S.tile([128, 1024], BF16, tag=f"h{k}", name=f"h{k}")
                      for k in range(3)]
            me_bufs = [S.tile([128, 1024], BF16, tag=f"me{k}", name=f"me{k}")
                       for k in range(NMEB)]
            tmp_bufs = [S.tile([128, 1024], F16, tag=f"tmp{k}",
                               name=f"tmp{k}") for k in range(2)]
            fd1_bufs = [S.tile([128, 512], F16, tag=f"fd1{k}",
                               name=f"fd1{k}") for k in range(2)]
            fd2_bufs = [S.tile([128, 256], F16, tag=f"fd2{k}",
                               name=f"fd2{k}") for k in range(2)]
            zb_bufs = [S.tile([128, 64], F32, tag=f"zb{k}", name=f"zb{k}")
                       for k in range(2)]
            th_bufs = [S.tile([128, 64], BF16, tag=f"th{k}", name=f"th{k}")
                       for k in range(2)]
            tp_bufs = [S.tile([128, 64], BF16, tag=f"tp{k}", name=f"tp{k}")
                       for k in range(2)]
            opad = S.tile([2, N, 6], F32, tag="opad")
            nc.vector.memset(opad, 0.0)

            def rear3(t, b):
                # [128, b*w] tile viewed as [128, b, w]
                return t.rearrange("p (b f) -> p b f", b=b)

            for mol in range(BM):
                feats = MP.tile([D, N], F32, tag="feats")
                nc.sync.dma_start(out=feats, in_=feats0[mol])
                rd = RDP.tile([16, 128, N], BF16, tag="rd")
                nc.sync.dma_start(out=rd, in_=rdin[mol])
                msk = MP.tile([D, N], F32, tag="msk")
                nc.sync.dma_start(out=msk, in_=mask12[mol])

                for lay in range(L):
                    fb = LP.tile([D, N], BF16, tag="fb")
                    nc.vector.tensor_copy(out=fb, in_=feats)
                    if lay > 0:
                        # refresh the replicated-feats rows of RD
                        for k in range(8):
                            nc.sync.dma_start(
                                out=rd[0:D, 16 * k:16 * (k + 1), :],
                                in_=bass.AP(
                                    tensor=fb.tensor, offset=fb.offset,
                                    ap=[list(fb.ap[0]), [0, 16], [1, N]]))

                    # ---- per-i constant matrix fjwb / fjT ----
                    ps_fj = PS.tile([128, 128], F32, tag="pa")
                    fe = fb.rearrange("p (c two) -> p two c", two=2)
                    nc.tensor.matmul(
                        ps_fj, lhsT=w1fjE_s[:, lay * 128:(lay + 1) * 128],
                        rhs=fe[:, 0, :], start=True, stop=False)
                    nc.tensor.matmul(
                        ps_fj, lhsT=w1fjO_s[:, lay * 128:(lay + 1) * 128],
                        rhs=fe[:, 1, :], start=False, stop=False)
                    nc.tensor.matmul(
                        ps_fj, lhsT=b1pad_s[:, lay * 128:(lay + 1) * 128],
                        rhs=onesr, start=False, stop=True)
                    fjwb = LP.tile([128, 128], F32, tag="fjwb")
                    nc.vector.tensor_copy(out=fjwb, in_=ps_fj)
                    ps_ft = PS.tile([128, 128], F32, tag="pa")
                    nc.tensor.transpose(ps_ft, fjwb, i128f_s)
                    fjT = LP.tile([128, 128], BF16, tag="fjT")
                    nc.vector.tensor_copy(out=fjT, in_=ps_ft)

                    # ---- LayerNorm of feats (feeds node MLP later) ----
                    ps_mu = PS.tile([1, N], F32, tag="pa")
                    nc.tensor.matmul(ps_mu, lhsT=c12_s, rhs=fb,
                                     start=True, stop=True)
                    stat = LP.tile([1, 2 * N], BF16, tag="stat")
                    nc.vector.tensor_copy(out=stat[:, 0:N], in_=ps_mu)
                    ps_bm = PS.tile([D, N], F32, tag="pa")
                    nc.tensor.matmul(ps_bm, lhsT=onesr[:, 0:D],
                                     rhs=stat[:, 0:N], start=True, stop=True)
                    ctr = LP.tile([D, N], BF16, tag="ctr")
                    nc.vector.tensor_sub(ctr, fb, ps_bm)
                    sqc = LP.tile([D, N], BF16, tag="sqc")
                    nc.vector.tensor_mul(sqc, ctr, ctr)
                    ps_v = PS.tile([1, N], F32, tag="pa")
                    nc.tensor.matmul(ps_v, lhsT=c12_s, rhs=sqc,
                                     start=True, stop=True)
                    sd = LP.tile([1, N], F32, tag="sd")
                    nc.scalar.activation(sd, ps_v, AF.Sqrt, bias=eps[:, 0:1])
                    with nc.allow_low_precision(reason="bf16 rstd"):
                        nc.vector.reciprocal(stat[:, N:2 * N], sd)
                    ps_br = PS.tile([D, N], F32, tag="pa")
                    nc.tensor.matmul(ps_br, lhsT=onesr[:, 0:D],
                                     rhs=stat[:, N:2 * N], start=True,
                                     stop=True)
                    nrm = LP.tile([D, N], BF16, tag="nrm")
                    nc.vector.tensor_mul(nrm, ctr, ps_br)
                    normed = LP.tile([D, N], F32, tag="normed")
                    nc.vector.tensor_scalar(
                        out=normed, in0=nrm,
                        scalar1=lng_s[:, lay:lay + 1],
                        scalar2=lnb_s[:, lay:lay + 1],
                        op0=ALU.mult, op1=ALU.add)

                    s1w_l = s1w_s[:, lay * 128:(lay + 1) * 128]
                    w2q_l = w2q_s[:, lay * 128:(lay + 1) * 128]
                    wg_l = wgrep_s[:, lay * 1024:(lay + 1) * 1024]
                    ps_mg = PG.tile([M, N], F32, tag="magg")

                    # software-pipelined pair loop: stage2+tail of pair k
                    # is emitted after stage1+h-act of pair k+1 so the ACT
                    # queue alternates h-act(k+1) / me-act(k) without
                    # waiting on the intervening stage2 matmuls.
                    def emit_stage1(pp):
                        h = h_bufs[pp % 3]
                        for half in range(2):
                            ph = PH.tile([128, 512], F32, tag="ph")
                            for q in range(2):
                                hl = half * 2 + q
                                s = 4 * pp + hl
                                sl = slice(q * N, (q + 1) * N)
                                nc.tensor.matmul(ph[:, sl], lhsT=s1w_l,
                                                 rhs=rd[:, s, :],
                                                 start=True, stop=False)
                                icol = i128b_s[:, s:s + 1]
                                ind_ap = bass.AP(
                                    tensor=icol.tensor, offset=icol.offset,
                                    ap=[list(icol.ap[0]), [0, N]])
                                nc.tensor.matmul(ph[:, sl], lhsT=fjT,
                                                 rhs=ind_ap,
                                                 start=False, stop=True)
                            nc.scalar.activation(
                                h[:, half * 512:(half + 1) * 512], ph,
                                AF.Silu)
                        return h

                    def emit_stage2(pp, h):
                        # 8 double-block matmuls: full-128 lhsT against the
                        # block-diagonal w2 pair -> even-i block in cols
                        # 0:64, odd-i in 64:128 (lhsT base stays 0:
                        # unchained matmuls with alternating lhsT partition
                        # bases hang the device).
                        pms = []
                        for half in range(2):
                            pm = PM.tile([128, 512], F32, tag="pm")
                            for q in range(2):
                                for ib in range(2):   # j half-block
                                    hl = half * 2 + q
                                    db = q * 2 + ib
                                    po = pm[:, db * 128:(db + 1) * 128]
                                    hs = h[:, hl * N + ib * 128:
                                           hl * N + (ib + 1) * 128]
                                    nc.tensor.matmul(po, lhsT=hs,
                                                     rhs=w2q_l,
                                                     start=True, stop=True)
                            pms.append(pm)
                        return pms

                    def emit_tail(pp, pms):
                        me = me_bufs[pp % NMEB]
                        for half in range(2):
                            nc.scalar.activation(
                                me[:, half * 512:(half + 1) * 512],
                                pms[half], AF.Silu)
                        # gate z: mult + fold tree (fp16)
                        tmp = tmp_bufs[pp % 2]
                        nc.vector.tensor_mul(tmp, me, wg_l)
                        t3 = rear3(tmp, 16)
                        fd1 = fd1_bufs[pp % 2]
                        nc.vector.tensor_tensor(
                            out=rear3(fd1, 16), in0=t3[:, :, 0:32],
                            in1=t3[:, :, 32:64], op=ALU.add)
                        f13 = rear3(fd1, 16)
                        fd2 = fd2_bufs[pp % 2]
                        nc.vector.tensor_tensor(
                            out=rear3(fd2, 16), in0=f13[:, :, 0:16],
                            in1=f13[:, :, 16:32], op=ALU.add)
                        zb = zb_bufs[(pp // 4) % 2]
                        nc.vector.tensor_reduce(
                            out=zb[:, (pp % 4) * 16:(pp % 4) * 16 + 16],
                            in_=rear3(fd2, 16), op=ALU.add, axis=X)
                        if pp % 4 == 3:
                            # gate + j-aggregation for this sg
                            sgi = pp // 4
                            zbs = zb_bufs[sgi % 2]
                            th = th_bufs[sgi % 2]
                            nc.scalar.activation(
                                th, zbs, AF.Tanh,
                                bias=gbh_s[:, lay:lay + 1], scale=0.5)
                            tp = tp_bufs[sgi % 2]
                            nc.vector.tensor_scalar_add(tp, th, 1.0)
                            pp0 = pp - 3
                            for q4 in range(4):
                                meq = me_bufs[(pp0 + q4) % NMEB]
                                for hl in range(4):
                                    for jj in range(2):
                                        i = 8 * (pp0 + q4) + 2 * hl + jj
                                        for ib in range(2):
                                            b = hl * 4 + ib * 2 + jj
                                            col = q4 * 16 + b
                                            nc.tensor.matmul(
                                                ps_mg[:, i:i + 1],
                                                lhsT=meq[:, b * M:
                                                         (b + 1) * M],
                                                rhs=tp[:, col:col + 1],
                                                start=(ib == 0),
                                                stop=(ib == 1))

                    pend = []
                    for pp in range(NPAIR):
                        h = emit_stage1(pp)
                        if len(pend) == 2:
                            emit_tail(*pend.pop(0))
                        pms = emit_stage2(pp, h)
                        pend.append((pp, pms))
                    for e in pend:
                        emit_tail(*e)

                    magg = LP.tile([M, N], F32, tag="magg")
                    nc.vector.tensor_copy(out=magg, in_=ps_mg)

                    # ---- node MLP + residual ----
                    ps_z1 = PS.tile([24, N], F32, tag="pa")
                    nc.tensor.matmul(ps_z1,
                                     lhsT=nw1a_s[:, lay * 24:(lay + 1) * 24],
                                     rhs=normed, start=True, stop=False)
                    nc.tensor.matmul(ps_z1,
                                     lhsT=nw1b_s[:, lay * 24:(lay + 1) * 24],
                                     rhs=magg, start=False, stop=True)
                    s1 = LP.tile([24, N], F32, tag="s1")
                    nc.scalar.activation(s1, ps_z1, AF.Silu,
                                         bias=nb1_s[:, lay:lay + 1])
                    ps_z2 = PS.tile([D, N], F32, tag="pa")
                    nc.tensor.matmul(ps_z2,
                                     lhsT=nw2_s[:, lay * D:(lay + 1) * D],
                                     rhs=s1, start=True, stop=True)
                    feats_new = MP.tile([D, N], F32, tag="feats")
                    nc.vector.scalar_tensor_tensor(
                        out=feats_new, in0=ps_z2,
                        scalar=nb2_s[:, lay:lay + 1], in1=feats,
                        op0=ALU.add, op1=ALU.add)
                    feats = feats_new

                # ---- final head ----
                fmask = MP.tile([D, N], F32, tag="fmask")
                nc.vector.tensor_mul(fmask, feats, msk)
                ps_r = PS.tile([M, N], F32, tag="pa")
                nc.tensor.matmul(ps_r, lhsT=mw1_s, rhs=fmask,
                                 start=True, stop=True)
                r1 = MP.tile([M, N], F32, tag="r1")
                nc.scalar.activation(r1, ps_r, AF.Relu, bias=mb1_s[:, 0:1])
                ps_o = PS.tile([2, N], F32, tag="pa")
                nc.tensor.matmul(ps_o, lhsT=mw2_s, rhs=r1,
                                 start=True, stop=True)
                nc.vector.tensor_scalar_add(opad[:, :, 0:1], ps_o,
                                            mb2_s[:, 0:1])
                nc.sync.dma_start(
                    out=out[mol].rearrange("n c k -> c n k"), in_=opad)

    nc.finalize()
    return nc


_NC = None


def _get_nc():
    global _NC
    if _NC is None:
        _NC = build_nc()
    return _NC


def _bf(a):
    return np.ascontiguousarray(np.asarray(a, np.float32).astype(
        ml_dtypes.bfloat16))


def _prep_maps(x, mask, edge_w1, edge_b1, edge_w2, edge_b2, gate_w, gate_b,
               ln_g, ln_b, node_w1, node_b1, node_w2, node_b2,
               mlp_w1, mlp_b1, mlp_w2, mlp_b2):
    f = np.float32
    x = np.asarray(x, f)
    maskf = np.asarray(mask, f)
    ew1 = np.asarray(edge_w1, f)          # [L, 25, 50]
    eb1 = np.asarray(edge_b1, f)          # [L, 50]
    ew2 = np.asarray(edge_w2, f) * LIP    # [L, 50, 64]
    eb2 = np.asarray(edge_b2, f)          # [L, 64]
    gw = np.asarray(gate_w, f) * LIP      # [L, 64, 1]
    gb = np.asarray(gate_b, f)            # [L, 1]

    # slots are i-pairs: the re-added per-i term uses ew1 rows 0:D, the
    # per-j matmul term uses ew1 rows D:2D (feats_j against W1's fj block)
    w1fj_h = ew1[:, D:2 * D, :]           # [L, 12, 50] per-j weights
    w1fi_h = ew1[:, 0:D, :]               # [L, 12, 50] per-i weights
    w1d = ew1[:, 2 * D, :]                # [L, 50]

    s1w_h = np.zeros((16, L, 128), f)
    w1fjE_h = np.zeros((D, L, 128), f)
    w1fjO_h = np.zeros((D, L, 128), f)
    b1pad_h = np.zeros((1, L, 128), f)
    w2q_h = np.zeros((128, L, 128), f)
    wgrep_h = np.zeros((128, L, 1024), f)
    gbh_h = np.zeros((128, L), f)
    for l in range(L):
        # stage1 combined lhsT: rows 0:12 feats_j weights (both quadrants),
        # rows 12:16 dist hi/lo x quadrant
        s1w_h[0:D, l, 0:EH] = w1fj_h[l]
        s1w_h[0:D, l, Q:Q + EH] = w1fj_h[l]
        s1w_h[12, l, 0:EH] = w1d[l]
        s1w_h[13, l, 0:EH] = w1d[l]
        s1w_h[14, l, Q:Q + EH] = w1d[l]
        s1w_h[15, l, Q:Q + EH] = w1d[l]
        # per-i matrix path (fjwb): i-even / i-odd quadrants + bias; col
        # EH/Q+EH carries the ones-row magnitude CONE for the b2 trick
        w1fjE_h[:, l, 0:EH] = w1fi_h[l]
        w1fjO_h[:, l, Q:Q + EH] = w1fi_h[l]
        b1pad_h[0, l, 0:EH] = eb1[l]
        b1pad_h[0, l, EH] = CONE
        b1pad_h[0, l, Q:Q + EH] = eb1[l]
        b1pad_h[0, l, Q + EH] = CONE
        # block-diagonal pair: even-i quadrant rows -> cols 0:64,
        # odd-i quadrant rows -> cols 64:128; row EH/Q+EH carries b2/CONE
        w2q_h[0:EH, l, 0:M] = ew2[l]
        w2q_h[EH, l, 0:M] = eb2[l] / CONE
        w2q_h[Q:Q + EH, l, M:2 * M] = ew2[l]
        w2q_h[Q + EH, l, M:2 * M] = eb2[l] / CONE
        wgrep_h[:, l, :] = np.tile(gw[l, :, 0], (128, 16))
        gbh_h[:, l] = gb[l, 0] * 0.5

    nw1 = np.asarray(node_w1, f)          # [L, 76, 24]
    nw1a_h = np.transpose(nw1[:, 0:D, :], (1, 0, 2))       # [12, L, 24]
    nw1b_h = np.transpose(nw1[:, D:, :] * (LIP * 0.5), (1, 0, 2))
    nw2_h = np.transpose(np.asarray(node_w2, f) * LIP, (1, 0, 2))

    parts = dict(
        s1w=_bf(s1w_h.reshape(16, L * 128)),
        w1fjE=_bf(w1fjE_h.reshape(D, L * 128)),
        w1fjO=_bf(w1fjO_h.reshape(D, L * 128)),
        b1pad=_bf(b1pad_h.reshape(1, L * 128)),
        w2q=_bf(w2q_h.reshape(128, L * 128)),
        wgrep=_bf(wgrep_h.reshape(128, L * 1024)),
        i128b=_bf(np.eye(128, dtype=f)),
        c12=_bf(np.full((D, 1), 1.0 / D, f)),
    )
    partsf = dict(
        gbh=gbh_h,
        lng=np.asarray(ln_g, f).T,
        lnb=np.asarray(ln_b, f).T,
        nw1a=nw1a_h.reshape(D, L * 24),
        nw1b=nw1b_h.reshape(M, L * 24),
        nb1=np.asarray(node_b1, f).T,
        nw2=nw2_h.reshape(24, L * D),
        nb2=np.asarray(node_b2, f).T,
        mw1=np.asarray(mlp_w1, f),
        mb1=np.asarray(mlp_b1, f).reshape(M, 1),
        mw2=np.asarray(mlp_w2, f),
        mb2=np.asarray(mlp_b2, f).reshape(2, 1),
        i128f=np.eye(128, dtype=f),
    )
    wbf_h = np.zeros((128, WBF_X), ml_dtypes.bfloat16)
    for nm, p, w in WBF_SPEC:
        wbf_h[0:p, WBF_OFF[nm][2]:WBF_OFF[nm][2] + w] = parts[nm]
    wf32_h = np.zeros((128, WF32_X), f)
    for nm, p, w in WF32_SPEC:
        wf32_h[0:p, WF32_OFF[nm][2]:WF32_OFF[nm][2] + w] = partsf[nm]
    shared = dict(wbf=wbf_h, wf32=wf32_h)

    in_maps = []
    for core in range(NCORES):
        xs = x[core * BM:(core + 1) * BM]          # [2, 256, 6]
        feats0_h = np.zeros((BM, D, N), f)
        rd_h = np.zeros((BM, 16, 128, N), np.float32)
        m12 = np.zeros((BM, D, N), f)
        for m in range(BM):
            xm = xs[m]                              # [256, 6]
            fcat = np.concatenate([xm, xm], axis=1).T   # [12, 256]
            feats0_h[m] = fcat
            # rows 0:12: layer-0 feats (bf16) replicated over the 128 slots
            fcat_bf = fcat.astype(ml_dtypes.bfloat16).astype(np.float32)
            rd_h[m, 0:D] = np.broadcast_to(fcat_bf[:, None, :],
                                           (D, 128, N))
            nsq = np.sum(xm * xm, axis=1)           # [256]
            dmat = nsq[:, None] + nsq[None, :] - 2.0 * (xm @ xm.T)
            # rows 12:16 (parity, hi/lo): rd[12+2p+q][s, j] = d(2s + p, j)
            dpc = dmat.reshape(128, 2, N).transpose(1, 0, 2)  # [p, s, j]
            dhi = dpc.astype(ml_dtypes.bfloat16).astype(np.float32)
            dlo = dpc - dhi
            rd_h[m, 12] = dhi[0]
            rd_h[m, 13] = dlo[0]
            rd_h[m, 14] = dhi[1]
            rd_h[m, 15] = dlo[1]
            m12[m] = np.broadcast_to(maskf[core * BM + m], (D, N))
        in_maps.append(dict(
            feats0=np.ascontiguousarray(feats0_h), rdin=_bf(rd_h),
            mask12=np.ascontiguousarray(m12),
            **{k: v.copy() for k, v in shared.items()},
        ))
    return in_maps


def kernel(**inputs):
    nc = _get_nc()
    in_maps = _prep_maps(**inputs)
    res = run_bass_kernel_spmd(nc, in_maps, core_ids=list(range(NCORES)))
    out = np.concatenate([r["out"] for r in res.results], axis=0)
    return out.astype(np.float32)


# revision 33
# speedup vs baseline: 1.2178x; 1.0600x over previous
"""EGNN (gnn_message_passing) Trainium2 Bass kernel, v3.

v2 -> v3 changes, driven by the TimelineSim cost model (matmul cost =
OUT free size x cycles/row, independent of K; ACT/DVE cost = max free
size; DVE 2x perf mode needs all-2-byte packed operands):
- b2 enters stage2 via an ones-row in h (ph row 50/114 is forced to 16
  through the b1pad path; silu(16) rounds to exactly 16 in bf16; W2 row
  50/114 = b2/16).  Kills the 8 K=1 b2 matmuls per chunk-pair (-32K PE
  rows per mol-layer).
- stage1 is 2 matmuls per slot instead of 3: the per-j term (W1fj.f_j)
  and the dist term share one matmul against a packed [16,128,N] rhs
  tile RD whose rows 0:12 are feats replicated over the 128 slots
  (layer 0 comes replicated from DRAM; layer 1 is an SBUF->SBUF
  broadcast DMA in 8 pieces) and rows 12:16 are the hi/lo dist rows.
- ACT ops are [128,1024] spanning 2 PSUM banks: one shared 3-buf PSUM
  pool (6 banks) alternates stage1 ph / stage2 pm tiles, so both silus
  run at 1024 free (halves the per-op ACT overhead).
- gate z uses a fp16 fold tree (mult, 2 folds, reduce-16) instead of
  mult + full reduce: ~340ns less DVE per pair.
Layout is otherwise v2's: slots are i-pairs (2 dest nodes x 256 j) with
the two parities in partition quadrants 0:50 / 64:114; stage2 is
transposed into edge-major [128 j, 64 f] blocks; gating and
j-aggregation ride the PE as K=128, out-free-1 matmuls.
"""

import numpy as np
import ml_dtypes

import concourse.bass as bass
import concourse.bacc as bacc
import concourse.mybir as mybir
from concourse.tile import TileContext
from concourse.bass_utils import run_bass_kernel_spmd

F32 = mybir.dt.float32
F16 = mybir.dt.float16
BF16 = mybir.dt.bfloat16
AF = mybir.ActivationFunctionType
ALU = mybir.AluOpType
X = mybir.AxisListType.X

LIP = 0.909
NCORES = 8
BM = 2            # molecules per core
N = 256           # nodes per molecule
L = 2             # layers
D = 12            # feature dim
M = 64            # message dim
EH = 50           # edge hidden
Q = 64            # partition quadrant stride for the j-odd half
NSLOT = 128       # i-pair slots per molecule-layer
NPAIR = 32        # slot quads (4 slots / [128,1024] tile)
NMEB = 6          # me sbuf ring depth (pairs)
CONE = 16.0       # ones-row magnitude: silu(16) == 16 exactly in bf16

WBF_SPEC = [
    ("s1w", 16, L * 128), ("w1fjE", D, L * 128), ("w1fjO", D, L * 128),
    ("b1pad", 1, L * 128), ("w2q", 128, L * 128), ("wgrep", 128, L * 1024),
    ("i128b", 128, 128), ("c12", D, 1),
]
WF32_SPEC = [
    ("gbh", 128, L), ("lng", D, L), ("lnb", D, L), ("nw1a", D, L * 24),
    ("nw1b", M, L * 24), ("nb1", 24, L), ("nw2", 24, L * D),
    ("nb2", D, L), ("mw1", D, M), ("mb1", M, 1), ("mw2", M, 2),
    ("mb2", 2, 1), ("i128f", 128, 128),
]


def _offsets(spec):
    out, off = {}, 0
    for nm, p, w in spec:
        out[nm] = (p, w, off)
        off += w
    return out, off


WBF_OFF, WBF_X = _offsets(WBF_SPEC)
WF32_OFF, WF32_X = _offsets(WF32_SPEC)


def build_nc():
    nc = bacc.Bacc("TRN2", target_bir_lowering=False, debug=False)

    feats0 = nc.dram_tensor("feats0", [BM, D, N], F32, kind="ExternalInput")
    rdin = nc.dram_tensor("rdin", [BM, 16, 128, N], BF16,
                          kind="ExternalInput")
    mask12 = nc.dram_tensor("mask12", [BM, D, N], F32, kind="ExternalInput")
    wbf = nc.dram_tensor("wbf", [128, WBF_X], BF16, kind="ExternalInput")
    wf32 = nc.dram_tensor("wf32", [128, WF32_X], F32, kind="ExternalInput")
    out = nc.dram_tensor("out", [BM, N, 2, 6], F32, kind="ExternalOutput")

    with TileContext(nc) as tc:
        with (
            tc.tile_pool(name="singles", bufs=1) as S,
            tc.tile_pool(name="mol", bufs=3) as MP,
            tc.tile_pool(name="rdp", bufs=2) as RDP,
            tc.tile_pool(name="lay", bufs=3) as LP,
            tc.tile_pool(name="ph", bufs=2, space="PSUM") as PH,
            tc.tile_pool(name="pm", bufs=2, space="PSUM") as PM,
            tc.tile_pool(name="psml", bufs=1, space="PSUM") as PS,
            tc.tile_pool(name="pg", bufs=1, space="PSUM") as PG,
        ):
            wbf_s = S.tile([128, WBF_X], BF16, tag="wbf", name="wbf")
            nc.sync.dma_start(out=wbf_s, in_=wbf[:, :])
            wf32_s = S.tile([128, WF32_X], F32, tag="wf32", name="wf32")
            nc.sync.dma_start(out=wf32_s, in_=wf32[:, :])

            def bsl(nm):
                p, w, off = WBF_OFF[nm]
                return wbf_s[0:p, off:off + w]

            def fsl(nm):
                p, w, off = WF32_OFF[nm]
                return wf32_s[0:p, off:off + w]

            s1w_s = bsl("s1w")
            w1fjE_s = bsl("w1fjE")
            w1fjO_s = bsl("w1fjO")
            b1pad_s = bsl("b1pad")
            w2q_s = bsl("w2q")
            wgrep_s = bsl("wgrep")
            i128b_s = bsl("i128b")
            c12_s = bsl("c12")
            gbh_s = fsl("gbh")
            lng_s = fsl("lng")
            lnb_s = fsl("lnb")
            nw1a_s = fsl("nw1a")
            nw1b_s = fsl("nw1b")
            nb1_s = fsl("nb1")
            nw2_s = fsl("nw2")
            nb2_s = fsl("nb2")
            mw1_s = fsl("mw1")
            mb1_s = fsl("mb1")
            mw2_s = fsl("mw2")
            mb2_s = fsl("mb2")
            i128f_s = fsl("i128f")

            onesr = S.tile([1, 128], BF16, tag="onesr")
            nc.vector.memset(onesr, 1.0)
            eps = S.tile([1, 1], F32, tag="eps")
            nc.vector.memset(eps, 1e-5)

            h_bufs = [S.tile([128, 1024], BF16, tag=f"h{k}", name=f"h{k}")
                      for k in range(3)]
            me_bufs = [S.tile([128, 1024], BF16, tag=f"me{k}", name=f"me{k}")
                       for k in range(NMEB)]
            tmp_bufs = [S.tile([128, 1024], F16, tag=f"tmp{k}",
                               name=f"tmp{k}") for k in range(2)]
            fd1_bufs = [S.tile([128, 512], F16, tag=f"fd1{k}",
                               name=f"fd1{k}") for k in range(2)]
            fd2_bufs = [S.tile([128, 256], F16, tag=f"fd2{k}",
                               name=f"fd2{k}") for k in range(2)]
            zb_bufs = [S.tile([128, 64], F32, tag=f"zb{k}", name=f"zb{k}")
                       for k in range(2)]
            th_bufs = [S.tile([128, 64], BF16, tag=f"th{k}", name=f"th{k}")
                       for k in range(2)]
            tp_bufs = [S.tile([128, 64], BF16, tag=f"tp{k}", name=f"tp{k}")
                       for k in range(2)]
            opad = S.tile([2, N, 6], F32, tag="opad")
            nc.vector.memset(opad, 0.0)

            def rear3(t, b):
                # [128, b*w] tile viewed as [128, b, w]
                return t.rearrange("p (b f) -> p b f", b=b)

            for mol in range(BM):
                feats = MP.tile([D, N], F32, tag="feats")
                nc.sync.dma_start(out=feats, in_=feats0[mol])
                rd = RDP.tile([16, 128, N], BF16, tag="rd")
                nc.sync.dma_start(out=rd, in_=rdin[mol])
                msk = MP.tile([D, N], F32, tag="msk")
                nc.sync.dma_start(out=msk, in_=mask12[mol])

                for lay in range(L):
                    fb = LP.tile([D, N], BF16, tag="fb")
                    nc.vector.tensor_copy(out=fb, in_=feats)
                    if lay > 0:
                        # refresh the replicated-feats rows of RD
                        for k in range(8):
                            nc.sync.dma_start(
                                out=rd[0:D, 16 * k:16 * (k + 1), :],
                                in_=bass.AP(
                                    tensor=fb.tensor, offset=fb.offset,
                                    ap=[list(fb.ap[0]), [0, 16], [1, N]]))

                    # ---- per-i constant matrix fjwb / fjT ----
                    ps_fj = PS.tile([128, 128], F32, tag="pa")
                    fe = fb.rearrange("p (c two) -> p two c", two=2)
                    nc.tensor.matmul(
                        ps_fj, lhsT=w1fjE_s[:, lay * 128:(lay + 1) * 128],
                        rhs=fe[:, 0, :], start=True, stop=False)
                    nc.tensor.matmul(
                        ps_fj, lhsT=w1fjO_s[:, lay * 128:(lay + 1) * 128],
                        rhs=fe[:, 1, :], start=False, stop=False)
                    nc.tensor.matmul(
                        ps_fj, lhsT=b1pad_s[:, lay * 128:(lay + 1) * 128],
                        rhs=onesr, start=False, stop=True)
                    fjwb = LP.tile([128, 128], F32, tag="fjwb")
                    nc.vector.tensor_copy(out=fjwb, in_=ps_fj)
                    ps_ft = PS.tile([128, 128], F32, tag="pa")
                    nc.tensor.transpose(ps_ft, fjwb, i128f_s)
                    fjT = LP.tile([128, 128], BF16, tag="fjT")
                    nc.vector.tensor_copy(out=fjT, in_=ps_ft)

                    # LayerNorm of feats (emitted mid pair-loop so its
                    # ACT-table swaps don't serialize the layer boundary)
                    lnbox = []

                    def emit_ln():
                        ps_mu = PS.tile([1, N], F32, tag="pa")
                        nc.tensor.matmul(ps_mu, lhsT=c12_s, rhs=fb,
                                         start=True, stop=True)
                        stat = LP.tile([1, 2 * N], BF16, tag="stat")
                        nc.vector.tensor_copy(out=stat[:, 0:N], in_=ps_mu)
                        ps_bm = PS.tile([D, N], F32, tag="pa")
                        nc.tensor.matmul(ps_bm, lhsT=onesr[:, 0:D],
                                         rhs=stat[:, 0:N], start=True,
                                         stop=True)
                        ctr = LP.tile([D, N], BF16, tag="ctr")
                        nc.vector.tensor_sub(ctr, fb, ps_bm)
                        sqc = LP.tile([D, N], BF16, tag="sqc")
                        nc.vector.tensor_mul(sqc, ctr, ctr)
                        ps_v = PS.tile([1, N], F32, tag="pa")
                        nc.tensor.matmul(ps_v, lhsT=c12_s, rhs=sqc,
                                         start=True, stop=True)
                        sd = LP.tile([1, N], F32, tag="sd")
                        nc.scalar.activation(sd, ps_v, AF.Sqrt,
                                             bias=eps[:, 0:1])
                        with nc.allow_low_precision(reason="bf16 rstd"):
                            nc.vector.reciprocal(stat[:, N:2 * N], sd)
                        ps_br = PS.tile([D, N], F32, tag="pa")
                        nc.tensor.matmul(ps_br, lhsT=onesr[:, 0:D],
                                         rhs=stat[:, N:2 * N], start=True,
                                         stop=True)
                        nrm = LP.tile([D, N], BF16, tag="nrm")
                        nc.vector.tensor_mul(nrm, ctr, ps_br)
                        normed = LP.tile([D, N], F32, tag="normed")
                        nc.vector.tensor_scalar(
                            out=normed, in0=nrm,
                            scalar1=lng_s[:, lay:lay + 1],
                            scalar2=lnb_s[:, lay:lay + 1],
                            op0=ALU.mult, op1=ALU.add)
                        lnbox.append(normed)

                    s1w_l = s1w_s[:, lay * 128:(lay + 1) * 128]
                    w2q_l = w2q_s[:, lay * 128:(lay + 1) * 128]
                    wg_l = wgrep_s[:, lay * 1024:(lay + 1) * 1024]
                    ps_mg = PG.tile([M, N], F32, tag="magg")

                    # software-pipelined pair loop: stage2+tail of pair k
                    # is emitted after stage1+h-act of pair k+1 so the ACT
                    # queue alternates h-act(k+1) / me-act(k) without
                    # waiting on the intervening stage2 matmuls.
                    def emit_stage1(pp):
                        h = h_bufs[pp % 3]
                        for half in range(2):
                            ph = PH.tile([128, 512], F32, tag="ph")
                            for q in range(2):
                                hl = half * 2 + q
                                s = 4 * pp + hl
                                sl = slice(q * N, (q + 1) * N)
                                nc.tensor.matmul(ph[:, sl], lhsT=s1w_l,
                                                 rhs=rd[:, s, :],
                                                 start=True, stop=False)
                                icol = i128b_s[:, s:s + 1]
                                ind_ap = bass.AP(
                                    tensor=icol.tensor, offset=icol.offset,
                                    ap=[list(icol.ap[0]), [0, N]])
                                nc.tensor.matmul(ph[:, sl], lhsT=fjT,
                                                 rhs=ind_ap,
                                                 start=False, stop=True)
                            nc.scalar.activation(
                                h[:, half * 512:(half + 1) * 512], ph,
                                AF.Silu)
                        return h

                    def emit_stage2(pp, h):
                        # 8 double-block matmuls: full-128 lhsT against the
                        # block-diagonal w2 pair -> even-i block in cols
                        # 0:64, odd-i in 64:128 (lhsT base stays 0:
                        # unchained matmuls with alternating lhsT partition
                        # bases hang the device).
                        pm = PM.tile([128, 1024], F32, tag="pm")
                        for hl in range(4):
                            for ib in range(2):   # j half-block
                                db = hl * 2 + ib
                                po = pm[:, db * 128:(db + 1) * 128]
                                hs = h[:, hl * N + ib * 128:
                                       hl * N + (ib + 1) * 128]
                                nc.tensor.matmul(po, lhsT=hs, rhs=w2q_l,
                                                 start=True, stop=True)
                        return pm

                    def emit_tail(pp, pm):
                        me = me_bufs[pp % NMEB]
                        nc.scalar.activation(me, pm, AF.Silu)
                        # gate z: mult + fold tree (fp16)
                        tmp = tmp_bufs[pp % 2]
                        nc.vector.tensor_mul(tmp, me, wg_l)
                        t3 = rear3(tmp, 16)
                        fd1 = fd1_bufs[pp % 2]
                        nc.vector.tensor_tensor(
                            out=rear3(fd1, 16), in0=t3[:, :, 0:32],
                            in1=t3[:, :, 32:64], op=ALU.add)
                        f13 = rear3(fd1, 16)
                        fd2 = fd2_bufs[pp % 2]
                        nc.vector.tensor_tensor(
                            out=rear3(fd2, 16), in0=f13[:, :, 0:16],
                            in1=f13[:, :, 16:32], op=ALU.add)
                        zb = zb_bufs[(pp // 4) % 2]
                        nc.vector.tensor_reduce(
                            out=zb[:, (pp % 4) * 16:(pp % 4) * 16 + 16],
                            in_=rear3(fd2, 16), op=ALU.add, axis=X)
                        if pp % 4 == 3:
                            # gate + j-aggregation for this sg
                            sgi = pp // 4
                            zbs = zb_bufs[sgi % 2]
                            th = th_bufs[sgi % 2]
                            nc.scalar.activation(
                                th, zbs, AF.Tanh,
                                bias=gbh_s[:, lay:lay + 1], scale=0.5)
                            tp = tp_bufs[sgi % 2]
                            nc.vector.tensor_scalar_add(tp, th, 1.0)
                            pp0 = pp - 3
                            for q4 in range(4):
                                meq = me_bufs[(pp0 + q4) % NMEB]
                                for hl in range(4):
                                    for jj in range(2):
                                        i = 8 * (pp0 + q4) + 2 * hl + jj
                                        for ib in range(2):
                                            b = hl * 4 + ib * 2 + jj
                                            col = q4 * 16 + b
                                            nc.tensor.matmul(
                                                ps_mg[:, i:i + 1],
                                                lhsT=meq[:, b * M:
                                                         (b + 1) * M],
                                                rhs=tp[:, col:col + 1],
                                                start=(ib == 0),
                                                stop=(ib == 1))

                    pend = []
                    for pp in range(NPAIR):
                        h = emit_stage1(pp)
                        if len(pend) == 2:
                            emit_tail(*pend.pop(0))
                        pm = emit_stage2(pp, h)
                        pend.append((pp, pm))
                        if pp == 2:
                            emit_ln()
                    for e in pend:
                        emit_tail(*e)
                    normed = lnbox[0]

                    magg = LP.tile([M, N], F32, tag="magg")
                    nc.vector.tensor_copy(out=magg, in_=ps_mg)

                    # ---- node MLP + residual ----
                    ps_z1 = PS.tile([24, N], F32, tag="pa")
                    nc.tensor.matmul(ps_z1,
                                     lhsT=nw1a_s[:, lay * 24:(lay + 1) * 24],
                                     rhs=normed, start=True, stop=False)
                    nc.tensor.matmul(ps_z1,
                                     lhsT=nw1b_s[:, lay * 24:(lay + 1) * 24],
                                     rhs=magg, start=False, stop=True)
                    s1 = LP.tile([24, N], F32, tag="s1")
                    nc.scalar.activation(s1, ps_z1, AF.Silu,
                                         bias=nb1_s[:, lay:lay + 1])
                    ps_z2 = PS.tile([D, N], F32, tag="pa")
                    nc.tensor.matmul(ps_z2,
                                     lhsT=nw2_s[:, lay * D:(lay + 1) * D],
                                     rhs=s1, start=True, stop=True)
                    feats_new = MP.tile([D, N], F32, tag="feats")
                    nc.vector.scalar_tensor_tensor(
                        out=feats_new, in0=ps_z2,
                        scalar=nb2_s[:, lay:lay + 1], in1=feats,
                        op0=ALU.add, op1=ALU.add)
                    feats = feats_new

                # ---- final head ----
                fmask = MP.tile([D, N], F32, tag="fmask")
                nc.vector.tensor_mul(fmask, feats, msk)
                ps_r = PS.tile([M, N], F32, tag="pa")
                nc.tensor.matmul(ps_r, lhsT=mw1_s, rhs=fmask,
                                 start=True, stop=True)
                r1 = MP.tile([M, N], F32, tag="r1")
                nc.scalar.activation(r1, ps_r, AF.Relu, bias=mb1_s[:, 0:1])
                ps_o = PS.tile([2, N], F32, tag="pa")
                nc.tensor.matmul(ps_o, lhsT=mw2_s, rhs=r1,
                                 start=True, stop=True)
                nc.vector.tensor_scalar_add(opad[:, :, 0:1], ps_o,
                                            mb2_s[:, 0:1])
                nc.sync.dma_start(
                    out=out[mol].rearrange("n c k -> c n k"), in_=opad)

    nc.finalize()
    return nc


_NC = None


def _get_nc():
    global _NC
    if _NC is None:
        _NC = build_nc()
    return _NC


def _bf(a):
    return np.ascontiguousarray(np.asarray(a, np.float32).astype(
        ml_dtypes.bfloat16))


def _prep_maps(x, mask, edge_w1, edge_b1, edge_w2, edge_b2, gate_w, gate_b,
               ln_g, ln_b, node_w1, node_b1, node_w2, node_b2,
               mlp_w1, mlp_b1, mlp_w2, mlp_b2):
    f = np.float32
    x = np.asarray(x, f)
    maskf = np.asarray(mask, f)
    ew1 = np.asarray(edge_w1, f)          # [L, 25, 50]
    eb1 = np.asarray(edge_b1, f)          # [L, 50]
    ew2 = np.asarray(edge_w2, f) * LIP    # [L, 50, 64]
    eb2 = np.asarray(edge_b2, f)          # [L, 64]
    gw = np.asarray(gate_w, f) * LIP      # [L, 64, 1]
    gb = np.asarray(gate_b, f)            # [L, 1]

    # slots are i-pairs: the re-added per-i term uses ew1 rows 0:D, the
    # per-j matmul term uses ew1 rows D:2D (feats_j against W1's fj block)
    w1fj_h = ew1[:, D:2 * D, :]           # [L, 12, 50] per-j weights
    w1fi_h = ew1[:, 0:D, :]               # [L, 12, 50] per-i weights
    w1d = ew1[:, 2 * D, :]                # [L, 50]

    s1w_h = np.zeros((16, L, 128), f)
    w1fjE_h = np.zeros((D, L, 128), f)
    w1fjO_h = np.zeros((D, L, 128), f)
    b1pad_h = np.zeros((1, L, 128), f)
    w2q_h = np.zeros((128, L, 128), f)
    wgrep_h = np.zeros((128, L, 1024), f)
    gbh_h = np.zeros((128, L), f)
    for l in range(L):
        # stage1 combined lhsT: rows 0:12 feats_j weights (both quadrants),
        # rows 12:16 dist hi/lo x quadrant
        s1w_h[0:D, l, 0:EH] = w1fj_h[l]
        s1w_h[0:D, l, Q:Q + EH] = w1fj_h[l]
        s1w_h[12, l, 0:EH] = w1d[l]
        s1w_h[13, l, 0:EH] = w1d[l]
        s1w_h[14, l, Q:Q + EH] = w1d[l]
        s1w_h[15, l, Q:Q + EH] = w1d[l]
        # per-i matrix path (fjwb): i-even / i-odd quadrants + bias; col
        # EH/Q+EH carries the ones-row magnitude CONE for the b2 trick
        w1fjE_h[:, l, 0:EH] = w1fi_h[l]
        w1fjO_h[:, l, Q:Q + EH] = w1fi_h[l]
        b1pad_h[0, l, 0:EH] = eb1[l]
        b1pad_h[0, l, EH] = CONE
        b1pad_h[0, l, Q:Q + EH] = eb1[l]
        b1pad_h[0, l, Q + EH] = CONE
        # block-diagonal pair: even-i quadrant rows -> cols 0:64,
        # odd-i quadrant rows -> cols 64:128; row EH/Q+EH carries b2/CONE
        w2q_h[0:EH, l, 0:M] = ew2[l]
        w2q_h[EH, l, 0:M] = eb2[l] / CONE
        w2q_h[Q:Q + EH, l, M:2 * M] = ew2[l]
        w2q_h[Q + EH, l, M:2 * M] = eb2[l] / CONE
        wgrep_h[:, l, :] = np.tile(gw[l, :, 0], (128, 16))
        gbh_h[:, l] = gb[l, 0] * 0.5

    nw1 = np.asarray(node_w1, f)          # [L, 76, 24]
    nw1a_h = np.transpose(nw1[:, 0:D, :], (1, 0, 2))       # [12, L, 24]
    nw1b_h = np.transpose(nw1[:, D:, :] * (LIP * 0.5), (1, 0, 2))
    nw2_h = np.transpose(np.asarray(node_w2, f) * LIP, (1, 0, 2))

    parts = dict(
        s1w=_bf(s1w_h.reshape(16, L * 128)),
        w1fjE=_bf(w1fjE_h.reshape(D, L * 128)),
        w1fjO=_bf(w1fjO_h.reshape(D, L * 128)),
        b1pad=_bf(b1pad_h.reshape(1, L * 128)),
        w2q=_bf(w2q_h.reshape(128, L * 128)),
        wgrep=_bf(wgrep_h.reshape(128, L * 1024)),
        i128b=_bf(np.eye(128, dtype=f)),
        c12=_bf(np.full((D, 1), 1.0 / D, f)),
    )
    partsf = dict(
        gbh=gbh_h,
        lng=np.asarray(ln_g, f).T,
        lnb=np.asarray(ln_b, f).T,
        nw1a=nw1a_h.reshape(D, L * 24),
        nw1b=nw1b_h.reshape(M, L * 24),
        nb1=np.asarray(node_b1, f).T,
        nw2=nw2_h.reshape(24, L * D),
        nb2=np.asarray(node_b2, f).T,
        mw1=np.asarray(mlp_w1, f),
        mb1=np.asarray(mlp_b1, f).reshape(M, 1),
        mw2=np.asarray(mlp_w2, f),
        mb2=np.asarray(mlp_b2, f).reshape(2, 1),
        i128f=np.eye(128, dtype=f),
    )
    wbf_h = np.zeros((128, WBF_X), ml_dtypes.bfloat16)
    for nm, p, w in WBF_SPEC:
        wbf_h[0:p, WBF_OFF[nm][2]:WBF_OFF[nm][2] + w] = parts[nm]
    wf32_h = np.zeros((128, WF32_X), f)
    for nm, p, w in WF32_SPEC:
        wf32_h[0:p, WF32_OFF[nm][2]:WF32_OFF[nm][2] + w] = partsf[nm]
    shared = dict(wbf=wbf_h, wf32=wf32_h)

    in_maps = []
    for core in range(NCORES):
        xs = x[core * BM:(core + 1) * BM]          # [2, 256, 6]
        feats0_h = np.zeros((BM, D, N), f)
        rd_h = np.zeros((BM, 16, 128, N), np.float32)
        m12 = np.zeros((BM, D, N), f)
        for m in range(BM):
            xm = xs[m]                              # [256, 6]
            fcat = np.concatenate([xm, xm], axis=1).T   # [12, 256]
            feats0_h[m] = fcat
            # rows 0:12: layer-0 feats (bf16) replicated over the 128 slots
            fcat_bf = fcat.astype(ml_dtypes.bfloat16).astype(np.float32)
            rd_h[m, 0:D] = np.broadcast_to(fcat_bf[:, None, :],
                                           (D, 128, N))
            nsq = np.sum(xm * xm, axis=1)           # [256]
            dmat = nsq[:, None] + nsq[None, :] - 2.0 * (xm @ xm.T)
            # rows 12:16 (parity, hi/lo): rd[12+2p+q][s, j] = d(2s + p, j)
            dpc = dmat.reshape(128, 2, N).transpose(1, 0, 2)  # [p, s, j]
            dhi = dpc.astype(ml_dtypes.bfloat16).astype(np.float32)
            dlo = dpc - dhi
            rd_h[m, 12] = dhi[0]
            rd_h[m, 13] = dlo[0]
            rd_h[m, 14] = dhi[1]
            rd_h[m, 15] = dlo[1]
            m12[m] = np.broadcast_to(maskf[core * BM + m], (D, N))
        in_maps.append(dict(
            feats0=np.ascontiguousarray(feats0_h), rdin=_bf(rd_h),
            mask12=np.ascontiguousarray(m12),
            **{k: v.copy() for k, v in shared.items()},
        ))
    return in_maps


def kernel(**inputs):
    nc = _get_nc()
    in_maps = _prep_maps(**inputs)
    res = run_bass_kernel_spmd(nc, in_maps, core_ids=list(range(NCORES)))
    out = np.concatenate([r["out"] for r in res.results], axis=0)
    return out.astype(np.float32)


# revision 34
# speedup vs baseline: 1.2316x; 1.0113x over previous
"""EGNN (gnn_message_passing) Trainium2 Bass kernel, v3.

v2 -> v3 changes, driven by the TimelineSim cost model (matmul cost =
OUT free size x cycles/row, independent of K; ACT/DVE cost = max free
size; DVE 2x perf mode needs all-2-byte packed operands):
- b2 enters stage2 via an ones-row in h (ph row 50/114 is forced to 16
  through the b1pad path; silu(16) rounds to exactly 16 in bf16; W2 row
  50/114 = b2/16).  Kills the 8 K=1 b2 matmuls per chunk-pair (-32K PE
  rows per mol-layer).
- stage1 is 2 matmuls per slot instead of 3: the per-j term (W1fj.f_j)
  and the dist term share one matmul against a packed [16,128,N] rhs
  tile RD whose rows 0:12 are feats replicated over the 128 slots
  (layer 0 comes replicated from DRAM; layer 1 is an SBUF->SBUF
  broadcast DMA in 8 pieces) and rows 12:16 are the hi/lo dist rows.
- ACT ops are [128,1024] spanning 2 PSUM banks: one shared 3-buf PSUM
  pool (6 banks) alternates stage1 ph / stage2 pm tiles, so both silus
  run at 1024 free (halves the per-op ACT overhead).
- gate z uses a fp16 fold tree (mult, 2 folds, reduce-16) instead of
  mult + full reduce: ~340ns less DVE per pair.
Layout is otherwise v2's: slots are i-pairs (2 dest nodes x 256 j) with
the two parities in partition quadrants 0:50 / 64:114; stage2 is
transposed into edge-major [128 j, 64 f] blocks; gating and
j-aggregation ride the PE as K=128, out-free-1 matmuls.
"""

import numpy as np
import ml_dtypes

import concourse.bass as bass
import concourse.bacc as bacc
import concourse.mybir as mybir
from concourse.tile import TileContext
from concourse.bass_utils import run_bass_kernel_spmd

F32 = mybir.dt.float32
F16 = mybir.dt.float16
BF16 = mybir.dt.bfloat16
AF = mybir.ActivationFunctionType
ALU = mybir.AluOpType
X = mybir.AxisListType.X

LIP = 0.909
NCORES = 8
BM = 2            # molecules per core
N = 256           # nodes per molecule
L = 2             # layers
D = 12            # feature dim
M = 64            # message dim
EH = 50           # edge hidden
Q = 64            # partition quadrant stride for the j-odd half
NSLOT = 128       # i-pair slots per molecule-layer
NPAIR = 32        # slot quads (4 slots / [128,1024] tile)
NMEB = 6          # me sbuf ring depth (pairs)
CONE = 16.0       # ones-row magnitude: silu(16) == 16 exactly in bf16

WBF_SPEC = [
    ("s1w", 16, L * 128), ("w1fjE", D, L * 128), ("w1fjO", D, L * 128),
    ("b1pad", 1, L * 128), ("w2q", 128, L * 128), ("wgrep", 128, L * 1024),
    ("i128b", 128, 128), ("c12", D, 1), ("nw1a", D, L * 24),
    ("nw1b", M, L * 24), ("nw2", 24, L * D), ("mw1", D, M), ("mw2", M, 2),
]
WF32_SPEC = [
    ("gbh", 128, L), ("lng", D, L), ("lnb", D, L), ("nb1", 24, L),
    ("nb2", D, L), ("mb1", M, 1), ("mb2", 2, 1), ("i128f", 128, 128),
]


def _offsets(spec):
    out, off = {}, 0
    for nm, p, w in spec:
        out[nm] = (p, w, off)
        off += w
    return out, off


WBF_OFF, WBF_X = _offsets(WBF_SPEC)
WF32_OFF, WF32_X = _offsets(WF32_SPEC)


def build_nc():
    nc = bacc.Bacc("TRN2", target_bir_lowering=False, debug=False)

    feats0 = nc.dram_tensor("feats0", [BM, D, N], F32, kind="ExternalInput")
    rdin = nc.dram_tensor("rdin", [BM, 16, 128, N], BF16,
                          kind="ExternalInput")
    mask12 = nc.dram_tensor("mask12", [BM, D, N], F32, kind="ExternalInput")
    wbf = nc.dram_tensor("wbf", [128, WBF_X], BF16, kind="ExternalInput")
    wf32 = nc.dram_tensor("wf32", [128, WF32_X], F32, kind="ExternalInput")
    out = nc.dram_tensor("out", [BM, N, 2, 6], F32, kind="ExternalOutput")

    with TileContext(nc) as tc:
        with (
            tc.tile_pool(name="singles", bufs=1) as S,
            tc.tile_pool(name="mol", bufs=3) as MP,
            tc.tile_pool(name="rdp", bufs=2) as RDP,
            tc.tile_pool(name="lay", bufs=3) as LP,
            tc.tile_pool(name="ph", bufs=2, space="PSUM") as PH,
            tc.tile_pool(name="pm", bufs=2, space="PSUM") as PM,
            tc.tile_pool(name="psml", bufs=1, space="PSUM") as PS,
            tc.tile_pool(name="pg", bufs=1, space="PSUM") as PG,
        ):
            wbf_s = S.tile([128, WBF_X], BF16, tag="wbf", name="wbf")
            nc.sync.dma_start(out=wbf_s, in_=wbf[:, :])
            wf32_s = S.tile([128, WF32_X], F32, tag="wf32", name="wf32")
            nc.sync.dma_start(out=wf32_s, in_=wf32[:, :])

            def bsl(nm):
                p, w, off = WBF_OFF[nm]
                return wbf_s[0:p, off:off + w]

            def fsl(nm):
                p, w, off = WF32_OFF[nm]
                return wf32_s[0:p, off:off + w]

            s1w_s = bsl("s1w")
            w1fjE_s = bsl("w1fjE")
            w1fjO_s = bsl("w1fjO")
            b1pad_s = bsl("b1pad")
            w2q_s = bsl("w2q")
            wgrep_s = bsl("wgrep")
            i128b_s = bsl("i128b")
            c12_s = bsl("c12")
            gbh_s = fsl("gbh")
            lng_s = fsl("lng")
            lnb_s = fsl("lnb")
            nw1a_s = bsl("nw1a")
            nw1b_s = bsl("nw1b")
            nb1_s = fsl("nb1")
            nw2_s = bsl("nw2")
            nb2_s = fsl("nb2")
            mw1_s = bsl("mw1")
            mb1_s = fsl("mb1")
            mw2_s = bsl("mw2")
            mb2_s = fsl("mb2")
            i128f_s = fsl("i128f")

            onesr = S.tile([1, 128], BF16, tag="onesr")
            nc.vector.memset(onesr, 1.0)
            eps = S.tile([1, 1], F32, tag="eps")
            nc.vector.memset(eps, 1e-5)

            h_bufs = [S.tile([128, 1024], BF16, tag=f"h{k}", name=f"h{k}")
                      for k in range(3)]
            me_bufs = [S.tile([128, 1024], BF16, tag=f"me{k}", name=f"me{k}")
                       for k in range(NMEB)]
            tmp_bufs = [S.tile([128, 1024], F16, tag=f"tmp{k}",
                               name=f"tmp{k}") for k in range(2)]
            fd1_bufs = [S.tile([128, 512], F16, tag=f"fd1{k}",
                               name=f"fd1{k}") for k in range(2)]
            fd2_bufs = [S.tile([128, 256], F16, tag=f"fd2{k}",
                               name=f"fd2{k}") for k in range(2)]
            zb_bufs = [S.tile([128, 64], F32, tag=f"zb{k}", name=f"zb{k}")
                       for k in range(2)]
            th_bufs = [S.tile([128, 64], BF16, tag=f"th{k}", name=f"th{k}")
                       for k in range(2)]
            tp_bufs = [S.tile([128, 64], BF16, tag=f"tp{k}", name=f"tp{k}")
                       for k in range(2)]
            opad = S.tile([2, N, 6], F32, tag="opad")
            nc.vector.memset(opad, 0.0)

            def rear3(t, b):
                # [128, b*w] tile viewed as [128, b, w]
                return t.rearrange("p (b f) -> p b f", b=b)

            for mol in range(BM):
                feats = MP.tile([D, N], F32, tag="feats")
                nc.sync.dma_start(out=feats, in_=feats0[mol])
                rd = RDP.tile([16, 128, N], BF16, tag="rd")
                nc.gpsimd.dma_start(out=rd, in_=rdin[mol])
                msk = MP.tile([D, N], F32, tag="msk")
                nc.sync.dma_start(out=msk, in_=mask12[mol])

                for lay in range(L):
                    fb = LP.tile([D, N], BF16, tag="fb")
                    nc.vector.tensor_copy(out=fb, in_=feats)
                    if lay > 0:
                        # refresh the replicated-feats rows of RD
                        for k in range(8):
                            nc.gpsimd.dma_start(
                                out=rd[0:D, 16 * k:16 * (k + 1), :],
                                in_=bass.AP(
                                    tensor=fb.tensor, offset=fb.offset,
                                    ap=[list(fb.ap[0]), [0, 16], [1, N]]))

                    # ---- per-i constant matrix fjwb / fjT ----
                    ps_fj = PS.tile([128, 128], F32, tag="pa")
                    fe = fb.rearrange("p (c two) -> p two c", two=2)
                    nc.tensor.matmul(
                        ps_fj, lhsT=w1fjE_s[:, lay * 128:(lay + 1) * 128],
                        rhs=fe[:, 0, :], start=True, stop=False)
                    nc.tensor.matmul(
                        ps_fj, lhsT=w1fjO_s[:, lay * 128:(lay + 1) * 128],
                        rhs=fe[:, 1, :], start=False, stop=False)
                    nc.tensor.matmul(
                        ps_fj, lhsT=b1pad_s[:, lay * 128:(lay + 1) * 128],
                        rhs=onesr, start=False, stop=True)
                    fjwb = LP.tile([128, 128], F32, tag="fjwb")
                    nc.vector.tensor_copy(out=fjwb, in_=ps_fj)
                    ps_ft = PS.tile([128, 128], F32, tag="pa")
                    nc.tensor.transpose(ps_ft, fjwb, i128f_s)
                    fjT = LP.tile([128, 128], BF16, tag="fjT")
                    nc.vector.tensor_copy(out=fjT, in_=ps_ft)

                    # LayerNorm of feats (emitted mid pair-loop so its
                    # ACT-table swaps don't serialize the layer boundary)
                    lnbox = []

                    def emit_ln():
                        ps_mu = PS.tile([1, N], F32, tag="pa")
                        nc.tensor.matmul(ps_mu, lhsT=c12_s, rhs=fb,
                                         start=True, stop=True)
                        stat = LP.tile([1, 2 * N], BF16, tag="stat")
                        nc.vector.tensor_copy(out=stat[:, 0:N], in_=ps_mu)
                        ps_bm = PS.tile([D, N], F32, tag="pa")
                        nc.tensor.matmul(ps_bm, lhsT=onesr[:, 0:D],
                                         rhs=stat[:, 0:N], start=True,
                                         stop=True)
                        ctr = LP.tile([D, N], BF16, tag="ctr")
                        nc.vector.tensor_sub(ctr, fb, ps_bm)
                        sqc = LP.tile([D, N], BF16, tag="sqc")
                        nc.vector.tensor_mul(sqc, ctr, ctr)
                        ps_v = PS.tile([1, N], F32, tag="pa")
                        nc.tensor.matmul(ps_v, lhsT=c12_s, rhs=sqc,
                                         start=True, stop=True)
                        sd = LP.tile([1, N], F32, tag="sd")
                        nc.scalar.activation(sd, ps_v, AF.Sqrt,
                                             bias=eps[:, 0:1])
                        with nc.allow_low_precision(reason="bf16 rstd"):
                            nc.vector.reciprocal(stat[:, N:2 * N], sd)
                        ps_br = PS.tile([D, N], F32, tag="pa")
                        nc.tensor.matmul(ps_br, lhsT=onesr[:, 0:D],
                                         rhs=stat[:, N:2 * N], start=True,
                                         stop=True)
                        nrm = LP.tile([D, N], BF16, tag="nrm")
                        nc.vector.tensor_mul(nrm, ctr, ps_br)
                        normed = LP.tile([D, N], BF16, tag="normed")
                        nc.vector.tensor_scalar(
                            out=normed, in0=nrm,
                            scalar1=lng_s[:, lay:lay + 1],
                            scalar2=lnb_s[:, lay:lay + 1],
                            op0=ALU.mult, op1=ALU.add)
                        lnbox.append(normed)

                    s1w_l = s1w_s[:, lay * 128:(lay + 1) * 128]
                    w2q_l = w2q_s[:, lay * 128:(lay + 1) * 128]
                    wg_l = wgrep_s[:, lay * 1024:(lay + 1) * 1024]
                    ps_mg = PG.tile([M, N], F32, tag="magg")

                    # software-pipelined pair loop: stage2+tail of pair k
                    # is emitted after stage1+h-act of pair k+1 so the ACT
                    # queue alternates h-act(k+1) / me-act(k) without
                    # waiting on the intervening stage2 matmuls.
                    def emit_stage1(pp):
                        h = h_bufs[pp % 3]
                        for half in range(2):
                            ph = PH.tile([128, 512], F32, tag="ph")
                            for q in range(2):
                                hl = half * 2 + q
                                s = 4 * pp + hl
                                sl = slice(q * N, (q + 1) * N)
                                nc.tensor.matmul(ph[:, sl], lhsT=s1w_l,
                                                 rhs=rd[:, s, :],
                                                 start=True, stop=False)
                                icol = i128b_s[:, s:s + 1]
                                ind_ap = bass.AP(
                                    tensor=icol.tensor, offset=icol.offset,
                                    ap=[list(icol.ap[0]), [0, N]])
                                nc.tensor.matmul(ph[:, sl], lhsT=fjT,
                                                 rhs=ind_ap,
                                                 start=False, stop=True)
                            nc.scalar.activation(
                                h[:, half * 512:(half + 1) * 512], ph,
                                AF.Silu)
                        return h

                    def emit_stage2(pp, h):
                        # 8 double-block matmuls: full-128 lhsT against the
                        # block-diagonal w2 pair -> even-i block in cols
                        # 0:64, odd-i in 64:128 (lhsT base stays 0:
                        # unchained matmuls with alternating lhsT partition
                        # bases hang the device).
                        pm = PM.tile([128, 1024], F32, tag="pm")
                        for hl in range(4):
                            for ib in range(2):   # j half-block
                                db = hl * 2 + ib
                                po = pm[:, db * 128:(db + 1) * 128]
                                hs = h[:, hl * N + ib * 128:
                                       hl * N + (ib + 1) * 128]
                                nc.tensor.matmul(po, lhsT=hs, rhs=w2q_l,
                                                 start=True, stop=True)
                        return pm

                    def emit_tail(pp, pm):
                        me = me_bufs[pp % NMEB]
                        nc.scalar.activation(me, pm, AF.Silu)
                        # gate z: mult + fold tree (fp16)
                        tmp = tmp_bufs[pp % 2]
                        nc.vector.tensor_mul(tmp, me, wg_l)
                        t3 = rear3(tmp, 16)
                        fd1 = fd1_bufs[pp % 2]
                        nc.vector.tensor_tensor(
                            out=rear3(fd1, 16), in0=t3[:, :, 0:32],
                            in1=t3[:, :, 32:64], op=ALU.add)
                        f13 = rear3(fd1, 16)
                        fd2 = fd2_bufs[pp % 2]
                        nc.vector.tensor_tensor(
                            out=rear3(fd2, 16), in0=f13[:, :, 0:16],
                            in1=f13[:, :, 16:32], op=ALU.add)
                        zb = zb_bufs[(pp // 4) % 2]
                        nc.vector.tensor_reduce(
                            out=zb[:, (pp % 4) * 16:(pp % 4) * 16 + 16],
                            in_=rear3(fd2, 16), op=ALU.add, axis=X)
                        if pp % 4 == 3:
                            # gate + j-aggregation for this sg
                            sgi = pp // 4
                            zbs = zb_bufs[sgi % 2]
                            th = th_bufs[sgi % 2]
                            nc.scalar.activation(
                                th, zbs, AF.Tanh,
                                bias=gbh_s[:, lay:lay + 1], scale=0.5)
                            tp = tp_bufs[sgi % 2]
                            nc.vector.tensor_scalar_add(tp, th, 1.0)
                            pp0 = pp - 3
                            for q4 in range(4):
                                meq = me_bufs[(pp0 + q4) % NMEB]
                                for hl in range(4):
                                    for jj in range(2):
                                        i = 8 * (pp0 + q4) + 2 * hl + jj
                                        for ib in range(2):
                                            b = hl * 4 + ib * 2 + jj
                                            col = q4 * 16 + b
                                            nc.tensor.matmul(
                                                ps_mg[:, i:i + 1],
                                                lhsT=meq[:, b * M:
                                                         (b + 1) * M],
                                                rhs=tp[:, col:col + 1],
                                                start=(ib == 0),
                                                stop=(ib == 1))

                    pend = []
                    for pp in range(NPAIR):
                        h = emit_stage1(pp)
                        if len(pend) == 2:
                            emit_tail(*pend.pop(0))
                        pm = emit_stage2(pp, h)
                        pend.append((pp, pm))
                        if pp == 6:
                            emit_ln()
                    for e in pend:
                        emit_tail(*e)
                    normed = lnbox[0]

                    magg = LP.tile([M, N], BF16, tag="magg")
                    nc.vector.tensor_copy(out=magg, in_=ps_mg)

                    # ---- node MLP + residual ----
                    ps_z1 = PS.tile([24, N], F32, tag="pa")
                    nc.tensor.matmul(ps_z1,
                                     lhsT=nw1a_s[:, lay * 24:(lay + 1) * 24],
                                     rhs=normed, start=True, stop=False)
                    nc.tensor.matmul(ps_z1,
                                     lhsT=nw1b_s[:, lay * 24:(lay + 1) * 24],
                                     rhs=magg, start=False, stop=True)
                    s1 = LP.tile([24, N], BF16, tag="s1")
                    nc.scalar.activation(s1, ps_z1, AF.Silu,
                                         bias=nb1_s[:, lay:lay + 1])
                    ps_z2 = PS.tile([D, N], F32, tag="pa")
                    nc.tensor.matmul(ps_z2,
                                     lhsT=nw2_s[:, lay * D:(lay + 1) * D],
                                     rhs=s1, start=True, stop=True)
                    feats_new = MP.tile([D, N], F32, tag="feats")
                    nc.vector.scalar_tensor_tensor(
                        out=feats_new, in0=ps_z2,
                        scalar=nb2_s[:, lay:lay + 1], in1=feats,
                        op0=ALU.add, op1=ALU.add)
                    feats = feats_new

                # ---- final head ----
                fmask = MP.tile([D, N], BF16, tag="fmask")
                nc.vector.tensor_mul(fmask, feats, msk)
                ps_r = PS.tile([M, N], F32, tag="pa")
                nc.tensor.matmul(ps_r, lhsT=mw1_s, rhs=fmask,
                                 start=True, stop=True)
                r1 = MP.tile([M, N], BF16, tag="r1")
                nc.scalar.activation(r1, ps_r, AF.Relu, bias=mb1_s[:, 0:1])
                ps_o = PS.tile([2, N], F32, tag="pa")
                nc.tensor.matmul(ps_o, lhsT=mw2_s, rhs=r1,
                                 start=True, stop=True)
                nc.vector.tensor_scalar_add(opad[:, :, 0:1], ps_o,
                                            mb2_s[:, 0:1])
                nc.sync.dma_start(
                    out=out[mol].rearrange("n c k -> c n k"), in_=opad)

    nc.finalize()
    return nc


_NC = None


def _get_nc():
    global _NC
    if _NC is None:
        _NC = build_nc()
    return _NC


def _bf(a):
    return np.ascontiguousarray(np.asarray(a, np.float32).astype(
        ml_dtypes.bfloat16))


def _prep_maps(x, mask, edge_w1, edge_b1, edge_w2, edge_b2, gate_w, gate_b,
               ln_g, ln_b, node_w1, node_b1, node_w2, node_b2,
               mlp_w1, mlp_b1, mlp_w2, mlp_b2):
    f = np.float32
    x = np.asarray(x, f)
    maskf = np.asarray(mask, f)
    ew1 = np.asarray(edge_w1, f)          # [L, 25, 50]
    eb1 = np.asarray(edge_b1, f)          # [L, 50]
    ew2 = np.asarray(edge_w2, f) * LIP    # [L, 50, 64]
    eb2 = np.asarray(edge_b2, f)          # [L, 64]
    gw = np.asarray(gate_w, f) * LIP      # [L, 64, 1]
    gb = np.asarray(gate_b, f)            # [L, 1]

    # slots are i-pairs: the re-added per-i term uses ew1 rows 0:D, the
    # per-j matmul term uses ew1 rows D:2D (feats_j against W1's fj block)
    w1fj_h = ew1[:, D:2 * D, :]           # [L, 12, 50] per-j weights
    w1fi_h = ew1[:, 0:D, :]               # [L, 12, 50] per-i weights
    w1d = ew1[:, 2 * D, :]                # [L, 50]

    s1w_h = np.zeros((16, L, 128), f)
    w1fjE_h = np.zeros((D, L, 128), f)
    w1fjO_h = np.zeros((D, L, 128), f)
    b1pad_h = np.zeros((1, L, 128), f)
    w2q_h = np.zeros((128, L, 128), f)
    wgrep_h = np.zeros((128, L, 1024), f)
    gbh_h = np.zeros((128, L), f)
    for l in range(L):
        # stage1 combined lhsT: rows 0:12 feats_j weights (both quadrants),
        # rows 12:16 dist hi/lo x quadrant
        s1w_h[0:D, l, 0:EH] = w1fj_h[l]
        s1w_h[0:D, l, Q:Q + EH] = w1fj_h[l]
        s1w_h[12, l, 0:EH] = w1d[l]
        s1w_h[13, l, 0:EH] = w1d[l]
        s1w_h[14, l, Q:Q + EH] = w1d[l]
        s1w_h[15, l, Q:Q + EH] = w1d[l]
        # per-i matrix path (fjwb): i-even / i-odd quadrants + bias; col
        # EH/Q+EH carries the ones-row magnitude CONE for the b2 trick
        w1fjE_h[:, l, 0:EH] = w1fi_h[l]
        w1fjO_h[:, l, Q:Q + EH] = w1fi_h[l]
        b1pad_h[0, l, 0:EH] = eb1[l]
        b1pad_h[0, l, EH] = CONE
        b1pad_h[0, l, Q:Q + EH] = eb1[l]
        b1pad_h[0, l, Q + EH] = CONE
        # block-diagonal pair: even-i quadrant rows -> cols 0:64,
        # odd-i quadrant rows -> cols 64:128; row EH/Q+EH carries b2/CONE
        w2q_h[0:EH, l, 0:M] = ew2[l]
        w2q_h[EH, l, 0:M] = eb2[l] / CONE
        w2q_h[Q:Q + EH, l, M:2 * M] = ew2[l]
        w2q_h[Q + EH, l, M:2 * M] = eb2[l] / CONE
        wgrep_h[:, l, :] = np.tile(gw[l, :, 0], (128, 16))
        gbh_h[:, l] = gb[l, 0] * 0.5

    nw1 = np.asarray(node_w1, f)          # [L, 76, 24]
    nw1a_h = np.transpose(nw1[:, 0:D, :], (1, 0, 2))       # [12, L, 24]
    nw1b_h = np.transpose(nw1[:, D:, :] * (LIP * 0.5), (1, 0, 2))
    nw2_h = np.transpose(np.asarray(node_w2, f) * LIP, (1, 0, 2))

    parts = dict(
        s1w=_bf(s1w_h.reshape(16, L * 128)),
        nw1a=_bf(nw1a_h.reshape(D, L * 24)),
        nw1b=_bf(nw1b_h.reshape(M, L * 24)),
        nw2=_bf(nw2_h.reshape(24, L * D)),
        mw1=_bf(np.asarray(mlp_w1, f)),
        mw2=_bf(np.asarray(mlp_w2, f)),
        w1fjE=_bf(w1fjE_h.reshape(D, L * 128)),
        w1fjO=_bf(w1fjO_h.reshape(D, L * 128)),
        b1pad=_bf(b1pad_h.reshape(1, L * 128)),
        w2q=_bf(w2q_h.reshape(128, L * 128)),
        wgrep=_bf(wgrep_h.reshape(128, L * 1024)),
        i128b=_bf(np.eye(128, dtype=f)),
        c12=_bf(np.full((D, 1), 1.0 / D, f)),
    )
    partsf = dict(
        gbh=gbh_h,
        lng=np.asarray(ln_g, f).T,
        lnb=np.asarray(ln_b, f).T,
        nb1=np.asarray(node_b1, f).T,
        nb2=np.asarray(node_b2, f).T,
        mb1=np.asarray(mlp_b1, f).reshape(M, 1),
        mb2=np.asarray(mlp_b2, f).reshape(2, 1),
        i128f=np.eye(128, dtype=f),
    )
    wbf_h = np.zeros((128, WBF_X), ml_dtypes.bfloat16)
    for nm, p, w in WBF_SPEC:
        wbf_h[0:p, WBF_OFF[nm][2]:WBF_OFF[nm][2] + w] = parts[nm]
    wf32_h = np.zeros((128, WF32_X), f)
    for nm, p, w in WF32_SPEC:
        wf32_h[0:p, WF32_OFF[nm][2]:WF32_OFF[nm][2] + w] = partsf[nm]
    shared = dict(wbf=wbf_h, wf32=wf32_h)

    in_maps = []
    for core in range(NCORES):
        xs = x[core * BM:(core + 1) * BM]          # [2, 256, 6]
        feats0_h = np.zeros((BM, D, N), f)
        rd_h = np.zeros((BM, 16, 128, N), np.float32)
        m12 = np.zeros((BM, D, N), f)
        for m in range(BM):
            xm = xs[m]                              # [256, 6]
            fcat = np.concatenate([xm, xm], axis=1).T   # [12, 256]
            feats0_h[m] = fcat
            # rows 0:12: layer-0 feats (bf16) replicated over the 128 slots
            fcat_bf = fcat.astype(ml_dtypes.bfloat16).astype(np.float32)
            rd_h[m, 0:D] = np.broadcast_to(fcat_bf[:, None, :],
                                           (D, 128, N))
            nsq = np.sum(xm * xm, axis=1)           # [256]
            dmat = nsq[:, None] + nsq[None, :] - 2.0 * (xm @ xm.T)
            # rows 12:16 (parity, hi/lo): rd[12+2p+q][s, j] = d(2s + p, j)
            dpc = dmat.reshape(128, 2, N).transpose(1, 0, 2)  # [p, s, j]
            dhi = dpc.astype(ml_dtypes.bfloat16).astype(np.float32)
            dlo = dpc - dhi
            rd_h[m, 12] = dhi[0]
            rd_h[m, 13] = dlo[0]
            rd_h[m, 14] = dhi[1]
            rd_h[m, 15] = dlo[1]
            m12[m] = np.broadcast_to(maskf[core * BM + m], (D, N))
        in_maps.append(dict(
            feats0=np.ascontiguousarray(feats0_h), rdin=_bf(rd_h),
            mask12=np.ascontiguousarray(m12),
            **{k: v.copy() for k, v in shared.items()},
        ))
    return in_maps


def kernel(**inputs):
    nc = _get_nc()
    in_maps = _prep_maps(**inputs)
    res = run_bass_kernel_spmd(nc, in_maps, core_ids=list(range(NCORES)))
    out = np.concatenate([r["out"] for r in res.results], axis=0)
    return out.astype(np.float32)
